# revision 16
# baseline (speedup 1.0000x reference)
"""Causal multi-head attention (B=4, T=2048, H=1024, 16 heads) on 8 trn2 cores.

Sharding: batch(4) x head-group(2).  Core c -> batch b=c//2, heads g=c%2
(8 heads each).  Each core computes its QKV projection slice, causal+padding
masked attention for its 8 heads, and a row-parallel slice of the output
projection.  The two partial outputs per batch row are summed on the host
(row-parallel unshard); b_out is folded in via a PSUM preload on one core's
output projection (the other core gets zeros).

Device algorithm (per core, attention kept transposed so softmax reduces
along the PE contraction dim):
  xT [H, T] (host-pretransposed input row)
  QT/KT [512, T] = wqk^T-slices @ xT   (Q pre-scaled by 1/sqrt(hd) on host)
  V    [T, 8x65]  = xT^T @ wv (+bias), bf16, ones column per head; rows with
                    key-padding are zeroed (incl. the ones col) -> padded keys
                    drop out of both the numerator and the softmax denominator.
  per (head pair, q-tile 512, k-chunk pair 2x128):
     S^T[k, q] = KT_h[:, kchunk].T @ QT_h[:, qtile]   (f32r, head pairs packed
                 into PE row groups 0-63 / 64-127 -> concurrent matmuls)
     (+ causal-mask PSUM preload via identity matmul on diagonal chunks)
     P^T = exp(S^T)              (ScalarE, [128,1024] two-chunk ops, bias 0)
     o^T[65, q] += V_aug[kchunk, head].T @ P^T        (row 64 = softmax denom)
  o_scaled = o^T[0:64] * (1/denom)  (DVE approx recip, gpsimd bcast), then
  DMA partition-shift into dense head-pair tiles [128, 512]
  y[t, j] = b_out (preload) + sum_hp o_dense_hp[:, t].T @ wout_hp[:, j]
"""

import os
import sys

import numpy as np

sys.path.insert(0, "/opt/trn_rl_repo")

B, T, H = 4, 2048, 1024
NH, HD = 16, 64
NCORES = 8
HPC = 8          # heads per core
GD = HPC * HD    # head dims per core = 512
KC = T // 128    # 16 k-chunks
QT_TILES = T // 512  # 4 q-tiles
HC = H // 128    # 8 h-chunks (contraction for projections)

NEG = -1.0e9


def _build_nc():
    import concourse.bass as bass
    import concourse.tile as tile
    import concourse.mybir as mybir
    from concourse import bacc
    from contextlib import ExitStack

    f32 = mybir.dt.float32
    f32r = mybir.dt.float32r
    bf16 = mybir.dt.bfloat16
    EXP = mybir.ActivationFunctionType.Exp

    nc = bacc.Bacc("TRN2", target_bir_lowering=False, debug=False)

    xT_d = nc.dram_tensor("xT", [H, T], f32, kind="ExternalInput").ap()
    wqk_d = nc.dram_tensor("wqk", [H, 2 * GD], f32, kind="ExternalInput").ap()
    wv_d = nc.dram_tensor("wv", [H, GD], f32, kind="ExternalInput").ap()
    bqk_d = nc.dram_tensor("bqk", [1, 2 * GD], f32, kind="ExternalInput").ap()
    bv_d = nc.dram_tensor("bv", [1, GD], f32, kind="ExternalInput").ap()
    wout_d = nc.dram_tensor("wout", [GD, H], f32, kind="ExternalInput").ap()
    bout_d = nc.dram_tensor("bout", [1, H], f32, kind="ExternalInput").ap()
    padb01_d = nc.dram_tensor("padb01", [128, KC], f32, kind="ExternalInput").ap()
    cmask_d = nc.dram_tensor("cmask", [128, 4 * 512], bf16, kind="ExternalInput").ap()
    ident_d = nc.dram_tensor("ident", [128, 128], bf16, kind="ExternalInput").ap()
    ones_d = nc.dram_tensor("ones", [1, 512], f32, kind="ExternalInput").ap()
    y_d = nc.dram_tensor("y", [T, H], f32, kind="ExternalOutput").ap()

    def r(ap):
        return ap.bitcast(f32r)

    with ExitStack() as ctx:
        tc = ctx.enter_context(tile.TileContext(nc))

        const = ctx.enter_context(tc.tile_pool(name="const", bufs=1))
        padb01_sb = const.tile([128, KC], f32, name="padb01_sb")
        nc.sync.dma_start(padb01_sb, padb01_d)
        ones_sb = const.tile([1, 512], f32, name="ones_sb")
        nc.sync.dma_start(r(ones_sb), r(ones_d))

        # Persistent activations
        acts = ctx.enter_context(tc.tile_pool(name="acts", bufs=1))
        qk_sb = [acts.tile([128, T], f32, name=f"qk{i}") for i in range(8)]
        v_sb = [acts.tile([128, HPC * 65], bf16, name=f"v{c}") for c in range(KC)]

        # ---------------- Phase 1: QKV projections ----------------
        with ExitStack() as p1:
            p1c = p1.enter_context(tc.tile_pool(name="p1c", bufs=1))
            bqk_sb = p1c.tile([1, 2 * GD], f32, name="bqk_sb")
            nc.sync.dma_start(r(bqk_sb), r(bqk_d))
            bv_sb = p1c.tile([1, GD], f32, name="bv_sb")
            nc.sync.dma_start(r(bv_sb), r(bv_d))

            xt_pool = p1.enter_context(tc.tile_pool(name="xt", bufs=1))
            xt = [xt_pool.tile([128, T], f32, name=f"xt{i}") for i in range(HC)]
            for i in range(HC):
                nc.sync.dma_start(r(xt[i]), r(xT_d[i * 128:(i + 1) * 128, :]))

            wqk_pool = p1.enter_context(tc.tile_pool(name="wqkp", bufs=16))
            wv_pool = p1.enter_context(tc.tile_pool(name="wvp", bufs=8))
            ps1 = p1.enter_context(tc.tile_pool(name="ps1", bufs=4, space="PSUM"))

            # Q^T and K^T: out[col, t] tiles
            for ct in range(8):
                wts = []
                for hc in range(HC):
                    wt = wqk_pool.tile([128, 128], f32, tag="w", name=f"w{ct}_{hc}")
                    nc.sync.dma_start(
                        r(wt), r(wqk_d[hc * 128:(hc + 1) * 128, ct * 128:(ct + 1) * 128]))
                    wts.append(wt)
                for tt in range(4):
                    ps = ps1.tile([128, 512], f32, tag="ps", name=f"psqk{ct}_{tt}")
                    nc.tensor.matmul(
                        ps, r(bqk_sb[0:1, ct * 128:(ct + 1) * 128]), r(ones_sb),
                        start=True, stop=False)
                    for hc in range(HC):
                        nc.tensor.matmul(
                            ps, r(wts[hc]), r(xt[hc][:, tt * 512:(tt + 1) * 512]),
                            start=False, stop=(hc == HC - 1))
                    nc.vector.tensor_copy(r(qk_sb[ct][:, tt * 512:(tt + 1) * 512]), ps)

            # V: out[t, col] tiles, bf16, interleaved [8 heads x 65], ones col;
            # all 65 cols multiplied by the key-padding 0/1 mask (per-partition).
            wvts = []
            for hc in range(HC):
                wvt = wv_pool.tile([128, GD], f32, tag="wv", name=f"wv{hc}")
                nc.sync.dma_start(r(wvt), r(wv_d[hc * 128:(hc + 1) * 128, :]))
                wvts.append(wvt)
            for ts in range(KC):
                psv = ps1.tile([128, 512], f32, tag="psv", name=f"psv{ts}")
                nc.tensor.matmul(psv, r(ones_sb[0:1, 0:128]), r(bv_sb),
                                 start=True, stop=False)
                for hc in range(HC):
                    nc.tensor.matmul(
                        psv, r(xt[hc][:, ts * 128:(ts + 1) * 128]), r(wvts[hc]),
                        start=False, stop=(hc == HC - 1))
                pad_c = padb01_sb[:, ts:ts + 1]
                dst = v_sb[ts].rearrange("p (h c) -> p h c", h=HPC)[:, :, 0:64]
                src = psv.rearrange("p (h c) -> p h c", h=HPC)
                nc.vector.tensor_scalar_mul(dst, src, pad_c)
                onescols = v_sb[ts].rearrange("p (h c) -> p h c", h=HPC)[:, :, 64:65]
                nc.vector.memset(onescols, 1.0)
                nc.vector.tensor_scalar_mul(onescols, onescols, pad_c)

        # ---------------- Phase 2: attention + output projection ----------------
        with ExitStack() as p2:
            p2c = p2.enter_context(tc.tile_pool(name="p2c", bufs=1))
            ident_sb = p2c.tile([128, 128], bf16, name="ident_sb")
            nc.sync.dma_start(ident_sb, ident_d)
            cmask_sb = p2c.tile([128, 4 * 512], bf16, name="cmask_sb")
            nc.sync.dma_start(cmask_sb, cmask_d)
            bout_sb = p2c.tile([1, H], f32, name="bout_sb")
            nc.sync.dma_start(r(bout_sb), r(bout_d))
            wout_sb = [p2c.tile([128, H], f32, name=f"wo{hp}") for hp in range(4)]
            for hp in range(4):
                nc.sync.dma_start(r(wout_sb[hp]), r(wout_d[hp * 128:(hp + 1) * 128, :]))

            ppool = p2.enter_context(tc.tile_pool(name="pchunks", bufs=12))
            osc_pool = p2.enter_context(tc.tile_pool(name="osc", bufs=2))
            oden_pool = p2.enter_context(tc.tile_pool(name="oden", bufs=8))
            dpool = p2.enter_context(tc.tile_pool(name="dtiles", bufs=2))
            ypool = p2.enter_context(tc.tile_pool(name="ysb", bufs=2))
            ps_s = p2.enter_context(tc.tile_pool(name="ps_s", bufs=2, space="PSUM"))
            ps_o = p2.enter_context(tc.tile_pool(name="ps_o", bufs=2, space="PSUM"))
            ps_y = p2.enter_context(tc.tile_pool(name="ps_y", bufs=2, space="PSUM"))

            def attn_tail(qt, h, opsum, o_dense):
                """softmax denom -> recip -> broadcast -> scale -> dense repack"""
                stage = dpool.tile([65, 512], f32, tag="dstage", name=f"st{qt}_{h}")
                nc.vector.tensor_copy(stage[64:65, :], opsum[64:65, :])
                dp0 = dpool.tile([1, 512], f32, tag="dp0", name=f"dp0_{qt}_{h}")
                nc.sync.dma_start(dp0, stage[64:65, :])
                rp0 = dpool.tile([1, 512], f32, tag="rp0", name=f"rp0_{qt}_{h}")
                nc.vector.reciprocal_approx_fast(rp0, dp0)
                rrep = dpool.tile([64, 512], f32, tag="rrep", name=f"rr{qt}_{h}")
                nc.gpsimd.partition_broadcast(rrep, rp0)
                o_sc = osc_pool.tile([64, 512], f32, tag="osc", name=f"osc{qt}_{h}")
                nc.vector.tensor_mul(o_sc, rrep, opsum[0:64, :])
                p0 = (h % 2) * 64
                nc.sync.dma_start(r(o_dense[p0:p0 + 64, :]), r(o_sc))

            def emit_y_tile(qt, j, ts, oden):
                """one output-projection tile for q-tile qt (b_out via preload)"""
                q0 = qt * 512
                ypsum = ps_y.tile([128, 512], f32, tag="y", name=f"y{qt}_{j}_{ts}")
                nc.tensor.matmul(
                    ypsum, r(ones_sb[0:1, 0:128]),
                    r(bout_sb[0:1, j * 512:(j + 1) * 512]),
                    start=True, stop=False)
                for hp in range(4):
                    nc.tensor.matmul(
                        ypsum,
                        r(oden[hp][:, ts * 128:(ts + 1) * 128]),
                        r(wout_sb[hp][:, j * 512:(j + 1) * 512]),
                        start=False, stop=(hp == 3))
                ysb = ypool.tile([128, 512], f32, tag="ysb", name=f"ys{qt}_{j}_{ts}")
                nc.vector.tensor_copy(ysb, ypsum)
                nc.sync.dma_start(
                    y_d[q0 + ts * 128:q0 + (ts + 1) * 128, j * 512:(j + 1) * 512],
                    ysb)

            pending_y = []   # deferred output-projection tiles of the prev q-tile

            for qt in range(QT_TILES):
                q0 = qt * 512
                nk = 4 * (qt + 1)
                oden = []
                for hp in range(4):
                    h0, h1 = 2 * hp, 2 * hp + 1
                    o_dense = oden_pool.tile([128, 512], f32, tag="od",
                                             name=f"od{qt}_{hp}")
                    oden.append(o_dense)
                    # S^T in two-chunk psum tiles; head pair packed into PE
                    # row groups via explicit tile_position (h0: rows 0-63,
                    # h1: rows 64-127) -> the two matmuls run concurrently.
                    # PV trails the exps by 2 chunk-groups so PE always has
                    # exp-independent work.
                    pts = {h0: [], h1: []}
                    opsum = {h: ps_o.tile([65, 512], f32, tag="o", name=f"o{qt}_{h}")
                             for h in (h0, h1)}

                    def emit_pv(h, cc):
                        for ci in range(2):
                            c = 2 * cc + ci
                            nc.tensor.matmul(
                                opsum[h],
                                v_sb[c][:, h * 65:(h + 1) * 65].bitcast(bf16),
                                pts[h][cc][:, ci * 512:ci * 512 + 512],
                                start=(c == 0), stop=(c == nk - 1))

                    for cc in range(nk // 2):
                        sps = {}
                        for h in (h0, h1):
                            sps[h] = ps_s.tile([128, 1024], f32, tag="s",
                                               name=f"s{qt}_{h}_{cc}")
                        for ci in range(2):
                            c = 2 * cc + ci
                            diag = c >= 4 * qt
                            dd = c - 4 * qt
                            if diag:
                                for h in (h0, h1):
                                    nc.tensor.matmul(
                                        sps[h][:, ci * 512:(ci + 1) * 512], ident_sb,
                                        cmask_sb[:, dd * 512:(dd + 1) * 512],
                                        start=True, stop=False)
                            for h in (h0, h1):
                                out = sps[h][:, ci * 512:(ci + 1) * 512]
                                hq = qk_sb[h // 2][(h % 2) * 64:(h % 2) * 64 + 64,
                                                   q0:q0 + 512]
                                hk = qk_sb[4 + h // 2][(h % 2) * 64:(h % 2) * 64 + 64,
                                                       c * 128:(c + 1) * 128]
                                nc.tensor.matmul(out, r(hk), r(hq),
                                                 start=not diag, stop=True,
                                                 tile_position=((h % 2) * 64, 0))
                        for h in (h0, h1):
                            pt = ppool.tile([128, 1024], bf16, tag="p",
                                            name=f"p{qt}_{h}_{cc}")
                            nc.scalar.activation(pt, sps[h], EXP, bias=0.0, scale=1.0)
                            pts[h].append(pt)
                        if pending_y:
                            pending_y.pop(0)()
                        if cc >= 2:
                            for h in (h0, h1):
                                emit_pv(h, cc - 2)
                    for cc in range(max(0, nk // 2 - 2), nk // 2):
                        for h in (h0, h1):
                            emit_pv(h, cc)
                    for h in (h0, h1):
                        attn_tail(qt, h, opsum[h], o_dense)

                for j in range(2):
                    for ts in range(4):
                        pending_y.append(
                            lambda qt=qt, j=j, ts=ts, oden=oden: emit_y_tile(qt, j, ts, oden))

            for fn in pending_y:
                fn()

    nc.compile()
    return nc


_NC_CACHE = None


def _get_nc():
    global _NC_CACHE
    if _NC_CACHE is None:
        _NC_CACHE = _build_nc()
    return _NC_CACHE


def make_core_inputs(input, mask, w_qkv, b_qkv, w_out, b_out, core):
    """Host-side sharding/layout prep for one core."""
    b, g = core // 2, core % 2
    scale = 1.0 / np.sqrt(HD)

    xT = np.ascontiguousarray(input[b].T).astype(np.float32)          # [H, T]

    qcols = slice(g * GD, (g + 1) * GD)
    kcols = slice(H + g * GD, H + (g + 1) * GD)
    vcols = slice(2 * H + g * GD, 2 * H + (g + 1) * GD)
    wq = w_qkv[:, qcols] * scale
    wk = w_qkv[:, kcols]
    wqk = np.ascontiguousarray(np.concatenate([wq, wk], axis=1)).astype(np.float32)
    bqk = np.concatenate([b_qkv[qcols] * scale, b_qkv[kcols]])[None, :].astype(np.float32)
    wv = np.ascontiguousarray(w_qkv[:, vcols]).astype(np.float32)
    bv = b_qkv[vcols][None, :].astype(np.float32)

    wout = np.ascontiguousarray(w_out[g * GD:(g + 1) * GD, :]).astype(np.float32)
    # b_out on core with g==0 only; zeros on g==1 (partials are summed on host)
    bout = (b_out if g == 0 else np.zeros_like(b_out))[None, :].astype(np.float32)

    padb01 = mask[b].astype(np.float32)                                # [T]
    padb01 = np.ascontiguousarray(padb01.reshape(KC, 128).T)           # [128, KC]

    # 4 causal diagonal mask patterns: delta = 128*dd; valid iff col >= row + delta
    import ml_dtypes
    cm = np.empty((128, 4 * 512), dtype=np.float32)
    rr = np.arange(128)[:, None]
    cc = np.arange(512)[None, :]
    for dd in range(4):
        cm[:, dd * 512:(dd + 1) * 512] = np.where(cc >= rr + 128 * dd, 0.0, NEG)
    cmask = cm.astype(ml_dtypes.bfloat16)
    ident = np.eye(128, dtype=np.float32).astype(ml_dtypes.bfloat16)
    ones = np.ones((1, 512), dtype=np.float32)

    return {
        "xT": xT, "wqk": wqk, "wv": wv, "bqk": bqk, "bv": bv,
        "wout": wout, "bout": bout, "padb01": padb01, "cmask": cmask,
        "ident": ident, "ones": ones,
    }


def kernel(input, mask, w_qkv, b_qkv, w_out, b_out):
    from concourse.bass_utils import run_bass_kernel_spmd

    nc = _get_nc()
    in_maps = [
        make_core_inputs(input, mask, w_qkv, b_qkv, w_out, b_out, c)
        for c in range(NCORES)
    ]
    res = run_bass_kernel_spmd(nc, in_maps, list(range(NCORES)))
    parts = [res.results[c]["y"] for c in range(NCORES)]
    out = np.stack([parts[2 * b] + parts[2 * b + 1] for b in range(B)])
    return out.astype(np.float32)


if __name__ == "__main__":
    nc = _build_nc()
    print("build ok")


# revision 17
# speedup vs baseline: 1.1508x; 1.1508x over previous
"""Causal multi-head attention (B=4, T=2048, H=1024, 16 heads) on 8 trn2 cores.

Sharding: batch(4) x head-group(2).  Core c -> batch b=c//2, heads g=c%2
(8 heads each).  Each core computes its QKV projection slice, causal+padding
masked attention for its 8 heads, and a row-parallel slice of the output
projection.  The two partial outputs per batch row are summed on the host
(row-parallel unshard); b_out is folded in via a PSUM preload on one core's
output projection (the other core gets zeros).

Device algorithm (per core, attention kept transposed so softmax reduces
along the PE contraction dim):
  xT [H, T] (host-pretransposed input row)
  QT/KT [512, T] = wqk^T-slices @ xT   (Q pre-scaled by 1/sqrt(hd) on host)
  V    [T, 8x65]  = xT^T @ wv (+bias), bf16, ones column per head; rows with
                    key-padding are zeroed (incl. the ones col) -> padded keys
                    drop out of both the numerator and the softmax denominator.
  per (head pair, q-tile 512, k-chunk pair 2x128):
     S^T[k, q] = KT_h[:, kchunk].T @ QT_h[:, qtile]   (f32r, head pairs packed
                 into PE row groups 0-63 / 64-127 -> concurrent matmuls)
     (+ causal-mask PSUM preload via identity matmul on diagonal chunks)
     P^T = exp(S^T)              (ScalarE, [128,1024] two-chunk ops, bias 0)
     o^T[65, q] += V_aug[kchunk, head].T @ P^T        (row 64 = softmax denom)
  o_scaled = o^T[0:64] * (1/denom)  (DVE approx recip, gpsimd bcast), then
  DMA partition-shift into dense head-pair tiles [128, 512]
  y[t, j] = b_out (preload) + sum_hp o_dense_hp[:, t].T @ wout_hp[:, j]
"""

import os
import sys

import numpy as np

sys.path.insert(0, "/opt/trn_rl_repo")

B, T, H = 4, 2048, 1024
NH, HD = 16, 64
NCORES = 8
HPC = 8          # heads per core
GD = HPC * HD    # head dims per core = 512
KC = T // 128    # 16 k-chunks
QT_TILES = T // 512  # 4 q-tiles
HC = H // 128    # 8 h-chunks (contraction for projections)

NEG = -1.0e9


def _build_nc():
    import concourse.bass as bass
    import concourse.tile as tile
    import concourse.mybir as mybir
    from concourse import bacc
    from contextlib import ExitStack

    f32 = mybir.dt.float32
    f32r = mybir.dt.float32r
    bf16 = mybir.dt.bfloat16
    EXP = mybir.ActivationFunctionType.Exp

    nc = bacc.Bacc("TRN2", target_bir_lowering=False, debug=False)

    xT_d = nc.dram_tensor("xT", [H, T], f32, kind="ExternalInput").ap()
    wqk_d = nc.dram_tensor("wqk", [H, 2 * GD], f32, kind="ExternalInput").ap()
    wv_d = nc.dram_tensor("wv", [H, GD], f32, kind="ExternalInput").ap()
    bqkc_d = nc.dram_tensor("bqkc", [128, 8], f32, kind="ExternalInput").ap()
    bv_d = nc.dram_tensor("bv", [1, GD], f32, kind="ExternalInput").ap()
    wout_d = nc.dram_tensor("wout", [GD, H], f32, kind="ExternalInput").ap()
    bout_d = nc.dram_tensor("bout", [1, H], f32, kind="ExternalInput").ap()
    padb01_d = nc.dram_tensor("padb01", [128, KC], f32, kind="ExternalInput").ap()
    cmask_d = nc.dram_tensor("cmask", [128, 4 * 512], bf16, kind="ExternalInput").ap()
    ident_d = nc.dram_tensor("ident", [128, 128], bf16, kind="ExternalInput").ap()
    ones_d = nc.dram_tensor("ones", [1, 512], f32, kind="ExternalInput").ap()
    y_d = nc.dram_tensor("y", [T, H], f32, kind="ExternalOutput").ap()

    def r(ap):
        return ap.bitcast(f32r)

    with ExitStack() as ctx:
        tc = ctx.enter_context(tile.TileContext(nc))

        const = ctx.enter_context(tc.tile_pool(name="const", bufs=1))
        padb01_sb = const.tile([128, KC], f32, name="padb01_sb")
        nc.sync.dma_start(padb01_sb, padb01_d)
        ones_sb = const.tile([1, 512], f32, name="ones_sb")
        nc.sync.dma_start(r(ones_sb), r(ones_d))

        # Persistent activations
        acts = ctx.enter_context(tc.tile_pool(name="acts", bufs=1))
        qk_sb = [acts.tile([128, T], f32, name=f"qk{i}") for i in range(8)]
        v_sb = [acts.tile([128, HPC * 65], bf16, name=f"v{c}") for c in range(KC)]

        # ---------------- Phase 1: QKV projections ----------------
        with ExitStack() as p1:
            p1c = p1.enter_context(tc.tile_pool(name="p1c", bufs=1))
            bqkc_sb = p1c.tile([128, 8], f32, name="bqkc_sb")
            nc.sync.dma_start(bqkc_sb, bqkc_d)
            bv_sb = p1c.tile([1, GD], f32, name="bv_sb")
            nc.sync.dma_start(r(bv_sb), r(bv_d))

            xt_pool = p1.enter_context(tc.tile_pool(name="xt", bufs=1))
            xt = [xt_pool.tile([128, T], f32, name=f"xt{i}") for i in range(HC)]
            for i in range(HC):
                nc.sync.dma_start(r(xt[i]), r(xT_d[i * 128:(i + 1) * 128, :]))

            wqk_pool = p1.enter_context(tc.tile_pool(name="wqkp", bufs=16))
            wv_pool = p1.enter_context(tc.tile_pool(name="wvp", bufs=8))
            ps1 = p1.enter_context(tc.tile_pool(name="ps1", bufs=4, space="PSUM"))

            # Q^T and K^T: out[col, t] tiles
            for ct in range(8):
                wts = []
                for hc in range(HC):
                    wt = wqk_pool.tile([128, 128], f32, tag="w", name=f"w{ct}_{hc}")
                    nc.sync.dma_start(
                        r(wt), r(wqk_d[hc * 128:(hc + 1) * 128, ct * 128:(ct + 1) * 128]))
                    wts.append(wt)
                for tt in range(4):
                    ps = ps1.tile([128, 512], f32, tag="ps", name=f"psqk{ct}_{tt}")
                    for hc in range(HC):
                        nc.tensor.matmul(
                            ps, r(wts[hc]), r(xt[hc][:, tt * 512:(tt + 1) * 512]),
                            start=(hc == 0), stop=(hc == HC - 1))
                    nc.vector.tensor_scalar_add(
                        r(qk_sb[ct][:, tt * 512:(tt + 1) * 512]), ps,
                        bqkc_sb[:, ct:ct + 1])

            # V: out[t, col] tiles, bf16, interleaved [8 heads x 65], ones col;
            # all 65 cols multiplied by the key-padding 0/1 mask (per-partition).
            wvts = []
            for hc in range(HC):
                wvt = wv_pool.tile([128, GD], f32, tag="wv", name=f"wv{hc}")
                nc.sync.dma_start(r(wvt), r(wv_d[hc * 128:(hc + 1) * 128, :]))
                wvts.append(wvt)
            for ts in range(KC):
                psv = ps1.tile([128, 512], f32, tag="psv", name=f"psv{ts}")
                nc.tensor.matmul(psv, r(ones_sb[0:1, 0:128]), r(bv_sb),
                                 start=True, stop=False)
                for hc in range(HC):
                    nc.tensor.matmul(
                        psv, r(xt[hc][:, ts * 128:(ts + 1) * 128]), r(wvts[hc]),
                        start=False, stop=(hc == HC - 1))
                pad_c = padb01_sb[:, ts:ts + 1]
                dst = v_sb[ts].rearrange("p (h c) -> p h c", h=HPC)[:, :, 0:64]
                src = psv.rearrange("p (h c) -> p h c", h=HPC)
                nc.vector.tensor_scalar_mul(dst, src, pad_c)
                onescols = v_sb[ts].rearrange("p (h c) -> p h c", h=HPC)[:, :, 64:65]
                nc.vector.memset(onescols, 1.0)
                nc.vector.tensor_scalar_mul(onescols, onescols, pad_c)

        # ---------------- Phase 2: attention + output projection ----------------
        with ExitStack() as p2:
            p2c = p2.enter_context(tc.tile_pool(name="p2c", bufs=1))
            ident_sb = p2c.tile([128, 128], bf16, name="ident_sb")
            nc.sync.dma_start(ident_sb, ident_d)
            cmask_sb = p2c.tile([128, 4 * 512], bf16, name="cmask_sb")
            nc.sync.dma_start(cmask_sb, cmask_d)
            bout_sb = p2c.tile([1, H], f32, name="bout_sb")
            nc.sync.dma_start(r(bout_sb), r(bout_d))
            wout_sb = [p2c.tile([128, H], f32, name=f"wo{hp}") for hp in range(4)]
            for hp in range(4):
                nc.sync.dma_start(r(wout_sb[hp]), r(wout_d[hp * 128:(hp + 1) * 128, :]))

            ppool = p2.enter_context(tc.tile_pool(name="pchunks", bufs=12))
            osc_pool = p2.enter_context(tc.tile_pool(name="osc", bufs=2))
            oden_pool = p2.enter_context(tc.tile_pool(name="oden", bufs=8))
            dpool = p2.enter_context(tc.tile_pool(name="dtiles", bufs=2))
            ypool = p2.enter_context(tc.tile_pool(name="ysb", bufs=2))
            ps_s = p2.enter_context(tc.tile_pool(name="ps_s", bufs=2, space="PSUM"))
            ps_o = p2.enter_context(tc.tile_pool(name="ps_o", bufs=2, space="PSUM"))
            ps_y = p2.enter_context(tc.tile_pool(name="ps_y", bufs=2, space="PSUM"))

            def attn_tail(qt, h, opsum, o_dense):
                """softmax denom -> recip -> broadcast -> scale -> dense repack"""
                stage = dpool.tile([65, 512], f32, tag="dstage", name=f"st{qt}_{h}")
                nc.vector.tensor_copy(stage[64:65, :], opsum[64:65, :])
                dp0 = dpool.tile([1, 512], f32, tag="dp0", name=f"dp0_{qt}_{h}")
                nc.sync.dma_start(dp0, stage[64:65, :])
                rp0 = dpool.tile([1, 512], f32, tag="rp0", name=f"rp0_{qt}_{h}")
                nc.vector.reciprocal_approx_fast(rp0, dp0)
                rrep = dpool.tile([64, 512], f32, tag="rrep", name=f"rr{qt}_{h}")
                nc.gpsimd.partition_broadcast(rrep, rp0)
                o_sc = osc_pool.tile([64, 512], f32, tag="osc", name=f"osc{qt}_{h}")
                nc.vector.tensor_mul(o_sc, rrep, opsum[0:64, :])
                p0 = (h % 2) * 64
                nc.sync.dma_start(r(o_dense[p0:p0 + 64, :]), r(o_sc))

            def emit_y_tile(qt, j, ts, oden):
                """one output-projection tile for q-tile qt (b_out via preload)"""
                q0 = qt * 512
                ypsum = ps_y.tile([128, 512], f32, tag="y", name=f"y{qt}_{j}_{ts}")
                nc.tensor.matmul(
                    ypsum, r(ones_sb[0:1, 0:128]),
                    r(bout_sb[0:1, j * 512:(j + 1) * 512]),
                    start=True, stop=False)
                for hp in range(4):
                    nc.tensor.matmul(
                        ypsum,
                        r(oden[hp][:, ts * 128:(ts + 1) * 128]),
                        r(wout_sb[hp][:, j * 512:(j + 1) * 512]),
                        start=False, stop=(hp == 3))
                ysb = ypool.tile([128, 512], f32, tag="ysb", name=f"ys{qt}_{j}_{ts}")
                nc.vector.tensor_copy(ysb, ypsum)
                nc.sync.dma_start(
                    y_d[q0 + ts * 128:q0 + (ts + 1) * 128, j * 512:(j + 1) * 512],
                    ysb)

            pending_y = []   # deferred output-projection tiles of the prev q-tile

            for qt in reversed(range(QT_TILES)):
                q0 = qt * 512
                nk = 4 * (qt + 1)
                oden = []
                for h in range(HPC):
                    if h % 2 == 0:
                        o_dense = oden_pool.tile([128, 512], f32, tag="od",
                                                 name=f"od{qt}_{h // 2}")
                        oden.append(o_dense)
                    hq = qk_sb[h // 2][(h % 2) * 64:(h % 2) * 64 + 64, q0:q0 + 512]
                    # S^T in two-chunk psum tiles, whole-head S stream first
                    # (exp trails on ScalarE with small frequent PE waits that
                    # don't trip the HAM throttle), then the dense PV stream.
                    pts = []
                    for cc in range(nk // 2):
                        spsum = ps_s.tile([128, 1024], f32, tag="s",
                                          name=f"s{qt}_{h}_{cc}")
                        for ci in range(2):
                            c = 2 * cc + ci
                            diag = c >= 4 * qt
                            dd = c - 4 * qt
                            out = spsum[:, ci * 512:(ci + 1) * 512]
                            if diag:
                                nc.tensor.matmul(
                                    out, ident_sb,
                                    cmask_sb[:, dd * 512:(dd + 1) * 512],
                                    start=True, stop=False)
                            hk = qk_sb[4 + h // 2][(h % 2) * 64:(h % 2) * 64 + 64,
                                                   c * 128:(c + 1) * 128]
                            nc.tensor.matmul(out, r(hk), r(hq),
                                             start=not diag, stop=True)
                        pt = ppool.tile([128, 1024], bf16, tag="p",
                                        name=f"p{qt}_{h}_{cc}")
                        nc.scalar.activation(pt, spsum, EXP, bias=0.0, scale=1.0)
                        pts.append(pt)
                        if cc == 1 and pending_y:
                            pending_y.pop(0)()
                    opsum = ps_o.tile([65, 512], f32, tag="o", name=f"o{qt}_{h}")
                    for c in range(nk):
                        nc.tensor.matmul(
                            opsum,
                            v_sb[c][:, h * 65:(h + 1) * 65].bitcast(bf16),
                            pts[c // 2][:, (c % 2) * 512:(c % 2) * 512 + 512],
                            start=(c == 0), stop=(c == nk - 1))
                    attn_tail(qt, h, opsum, o_dense)

                for j in range(2):
                    for ts in range(4):
                        pending_y.append(
                            lambda qt=qt, j=j, ts=ts, oden=oden: emit_y_tile(qt, j, ts, oden))

            for fn in pending_y:
                fn()

    nc.compile()
    return nc


_NC_CACHE = None


def _get_nc():
    global _NC_CACHE
    if _NC_CACHE is None:
        _NC_CACHE = _build_nc()
    return _NC_CACHE


def make_core_inputs(input, mask, w_qkv, b_qkv, w_out, b_out, core):
    """Host-side sharding/layout prep for one core."""
    b, g = core // 2, core % 2
    scale = 1.0 / np.sqrt(HD)

    xT = np.ascontiguousarray(input[b].T).astype(np.float32)          # [H, T]

    qcols = slice(g * GD, (g + 1) * GD)
    kcols = slice(H + g * GD, H + (g + 1) * GD)
    vcols = slice(2 * H + g * GD, 2 * H + (g + 1) * GD)
    wq = w_qkv[:, qcols] * scale
    wk = w_qkv[:, kcols]
    wqk = np.ascontiguousarray(np.concatenate([wq, wk], axis=1)).astype(np.float32)
    bqk = np.concatenate([b_qkv[qcols] * scale, b_qkv[kcols]]).astype(np.float32)
    bqkc = np.ascontiguousarray(bqk.reshape(8, 128).T)               # [128, 8]
    wv = np.ascontiguousarray(w_qkv[:, vcols]).astype(np.float32)
    bv = b_qkv[vcols][None, :].astype(np.float32)

    wout = np.ascontiguousarray(w_out[g * GD:(g + 1) * GD, :]).astype(np.float32)
    # b_out on core with g==0 only; zeros on g==1 (partials are summed on host)
    bout = (b_out if g == 0 else np.zeros_like(b_out))[None, :].astype(np.float32)

    padb01 = mask[b].astype(np.float32)                                # [T]
    padb01 = np.ascontiguousarray(padb01.reshape(KC, 128).T)           # [128, KC]

    # 4 causal diagonal mask patterns: delta = 128*dd; valid iff col >= row + delta
    import ml_dtypes
    cm = np.empty((128, 4 * 512), dtype=np.float32)
    rr = np.arange(128)[:, None]
    cc = np.arange(512)[None, :]
    for dd in range(4):
        cm[:, dd * 512:(dd + 1) * 512] = np.where(cc >= rr + 128 * dd, 0.0, NEG)
    cmask = cm.astype(ml_dtypes.bfloat16)
    ident = np.eye(128, dtype=np.float32).astype(ml_dtypes.bfloat16)
    ones = np.ones((1, 512), dtype=np.float32)

    return {
        "xT": xT, "wqk": wqk, "wv": wv, "bqkc": bqkc, "bv": bv,
        "wout": wout, "bout": bout, "padb01": padb01, "cmask": cmask,
        "ident": ident, "ones": ones,
    }


def kernel(input, mask, w_qkv, b_qkv, w_out, b_out):
    from concourse.bass_utils import run_bass_kernel_spmd

    nc = _get_nc()
    in_maps = [
        make_core_inputs(input, mask, w_qkv, b_qkv, w_out, b_out, c)
        for c in range(NCORES)
    ]
    res = run_bass_kernel_spmd(nc, in_maps, list(range(NCORES)))
    parts = [res.results[c]["y"] for c in range(NCORES)]
    out = np.stack([parts[2 * b] + parts[2 * b + 1] for b in range(B)])
    return out.astype(np.float32)


if __name__ == "__main__":
    nc = _build_nc()
    print("build ok")


# revision 19
# speedup vs baseline: 1.3036x; 1.1328x over previous
"""Causal multi-head attention (B=4, T=2048, H=1024, 16 heads) on 8 trn2 cores.

Sharding: batch(4) x head-group(2).  Core c -> batch b=c//2, heads g=c%2
(8 heads each).  Each core computes its QKV projection slice, causal+padding
masked attention for its 8 heads, and a row-parallel slice of the output
projection.  The two partial outputs per batch row are summed on the host
(row-parallel unshard); b_out is folded in via a PSUM preload on one core's
output projection (the other core gets zeros).

Device algorithm (per core, attention kept transposed so softmax reduces
along the PE contraction dim):
  xT [H, T] (host-pretransposed input row)
  QT/KT [512, T] = wqk^T-slices @ xT   (Q pre-scaled by 1/sqrt(hd) on host)
  V    [T, 8x65]  = xT^T @ wv (+bias), bf16, ones column per head; rows with
                    key-padding are zeroed (incl. the ones col) -> padded keys
                    drop out of both the numerator and the softmax denominator.
  per (head pair, q-tile 512, k-chunk pair 2x128):
     S^T[k, q] = KT_h[:, kchunk].T @ QT_h[:, qtile]   (f32r, head pairs packed
                 into PE row groups 0-63 / 64-127 -> concurrent matmuls)
     (+ causal-mask PSUM preload via identity matmul on diagonal chunks)
     P^T = exp(S^T)              (ScalarE, [128,1024] two-chunk ops, bias 0)
     o^T[65, q] += V_aug[kchunk, head].T @ P^T        (row 64 = softmax denom)
  o_scaled = o^T[0:64] * (1/denom)  (DVE approx recip, gpsimd bcast), then
  DMA partition-shift into dense head-pair tiles [128, 512]
  y[t, j] = b_out (preload) + sum_hp o_dense_hp[:, t].T @ wout_hp[:, j]
"""

import os
import sys

import numpy as np

sys.path.insert(0, "/opt/trn_rl_repo")

B, T, H = 4, 2048, 1024
NH, HD = 16, 64
NCORES = 8
HPC = 8          # heads per core
GD = HPC * HD    # head dims per core = 512
KC = T // 128    # 16 k-chunks
QT_TILES = T // 512  # 4 q-tiles
HC = H // 128    # 8 h-chunks (contraction for projections)

NEG = -1.0e9


def _build_nc():
    import concourse.bass as bass
    import concourse.tile as tile
    import concourse.mybir as mybir
    from concourse import bacc
    from contextlib import ExitStack

    f32 = mybir.dt.float32
    f32r = mybir.dt.float32r
    bf16 = mybir.dt.bfloat16
    EXP = mybir.ActivationFunctionType.Exp

    nc = bacc.Bacc("TRN2", target_bir_lowering=False, debug=False)

    xT_d = nc.dram_tensor("xT", [H, T], f32, kind="ExternalInput").ap()
    wqk_d = nc.dram_tensor("wqk", [H, 2 * GD], f32, kind="ExternalInput").ap()
    wv_d = nc.dram_tensor("wv", [H, GD], f32, kind="ExternalInput").ap()
    bqkc_d = nc.dram_tensor("bqkc", [128, 8], f32, kind="ExternalInput").ap()
    bv_d = nc.dram_tensor("bv", [1, GD], f32, kind="ExternalInput").ap()
    wout_d = nc.dram_tensor("wout", [GD, H], f32, kind="ExternalInput").ap()
    bout_d = nc.dram_tensor("bout", [1, H], f32, kind="ExternalInput").ap()
    padb01_d = nc.dram_tensor("padb01", [128, KC], f32, kind="ExternalInput").ap()
    cmask_d = nc.dram_tensor("cmask", [128, 4 * 512], bf16, kind="ExternalInput").ap()
    ident_d = nc.dram_tensor("ident", [128, 128], bf16, kind="ExternalInput").ap()
    ones_d = nc.dram_tensor("ones", [1, 512], f32, kind="ExternalInput").ap()
    y_d = nc.dram_tensor("y", [T, H], f32, kind="ExternalOutput").ap()

    def r(ap):
        return ap.bitcast(f32r)

    with ExitStack() as ctx:
        tc = ctx.enter_context(tile.TileContext(nc))

        const = ctx.enter_context(tc.tile_pool(name="const", bufs=1))
        padb01_sb = const.tile([128, KC], f32, name="padb01_sb")
        nc.sync.dma_start(padb01_sb, padb01_d)
        ones_sb = const.tile([1, 512], f32, name="ones_sb")
        nc.sync.dma_start(r(ones_sb), r(ones_d))

        # Persistent activations
        acts = ctx.enter_context(tc.tile_pool(name="acts", bufs=1))
        qk_sb = [acts.tile([128, T], bf16, name=f"qk{i}") for i in range(8)]
        v_sb = [acts.tile([128, HPC * 65], bf16, name=f"v{c}") for c in range(KC)]

        # ---------------- Phase 1: QKV projections ----------------
        with ExitStack() as p1:
            p1c = p1.enter_context(tc.tile_pool(name="p1c", bufs=1))
            bqkc_sb = p1c.tile([128, 8], f32, name="bqkc_sb")
            nc.sync.dma_start(bqkc_sb, bqkc_d)
            bv_sb = p1c.tile([1, GD], f32, name="bv_sb")
            nc.sync.dma_start(r(bv_sb), r(bv_d))

            xt_pool = p1.enter_context(tc.tile_pool(name="xt", bufs=1))
            xt = [xt_pool.tile([128, T], f32, name=f"xt{i}") for i in range(HC)]
            for i in range(HC):
                nc.sync.dma_start(r(xt[i]), r(xT_d[i * 128:(i + 1) * 128, :]))

            wqk_pool = p1.enter_context(tc.tile_pool(name="wqkp", bufs=16))
            wv_pool = p1.enter_context(tc.tile_pool(name="wvp", bufs=8))
            ps1 = p1.enter_context(tc.tile_pool(name="ps1", bufs=4, space="PSUM"))

            # Q^T and K^T: out[col, t] tiles
            for ct in range(8):
                wts = []
                for hc in range(HC):
                    wt = wqk_pool.tile([128, 128], f32, tag="w", name=f"w{ct}_{hc}")
                    nc.sync.dma_start(
                        r(wt), r(wqk_d[hc * 128:(hc + 1) * 128, ct * 128:(ct + 1) * 128]))
                    wts.append(wt)
                for tt in range(4):
                    ps = ps1.tile([128, 512], f32, tag="ps", name=f"psqk{ct}_{tt}")
                    for hc in range(HC):
                        nc.tensor.matmul(
                            ps, r(wts[hc]), r(xt[hc][:, tt * 512:(tt + 1) * 512]),
                            start=(hc == 0), stop=(hc == HC - 1))
                    nc.vector.tensor_scalar_add(
                        qk_sb[ct][:, tt * 512:(tt + 1) * 512], ps,
                        bqkc_sb[:, ct:ct + 1])

            # V: out[t, col] tiles, bf16, interleaved [8 heads x 65], ones col;
            # all 65 cols multiplied by the key-padding 0/1 mask (per-partition).
            wvts = []
            for hc in range(HC):
                wvt = wv_pool.tile([128, GD], f32, tag="wv", name=f"wv{hc}")
                nc.sync.dma_start(r(wvt), r(wv_d[hc * 128:(hc + 1) * 128, :]))
                wvts.append(wvt)
            for ts in range(KC):
                psv = ps1.tile([128, 512], f32, tag="psv", name=f"psv{ts}")
                nc.tensor.matmul(psv, r(ones_sb[0:1, 0:128]), r(bv_sb),
                                 start=True, stop=False)
                for hc in range(HC):
                    nc.tensor.matmul(
                        psv, r(xt[hc][:, ts * 128:(ts + 1) * 128]), r(wvts[hc]),
                        start=False, stop=(hc == HC - 1))
                pad_c = padb01_sb[:, ts:ts + 1]
                dst = v_sb[ts].rearrange("p (h c) -> p h c", h=HPC)[:, :, 0:64]
                src = psv.rearrange("p (h c) -> p h c", h=HPC)
                nc.vector.tensor_scalar_mul(dst, src, pad_c)
                onescols = v_sb[ts].rearrange("p (h c) -> p h c", h=HPC)[:, :, 64:65]
                nc.vector.memset(onescols, 1.0)
                nc.vector.tensor_scalar_mul(onescols, onescols, pad_c)

        # ---------------- Phase 2: attention + output projection ----------------
        with ExitStack() as p2:
            p2c = p2.enter_context(tc.tile_pool(name="p2c", bufs=1))
            ident_sb = p2c.tile([128, 128], bf16, name="ident_sb")
            nc.sync.dma_start(ident_sb, ident_d)
            cmask_sb = p2c.tile([128, 4 * 512], bf16, name="cmask_sb")
            nc.sync.dma_start(cmask_sb, cmask_d)
            bout_sb = p2c.tile([1, H], f32, name="bout_sb")
            nc.sync.dma_start(r(bout_sb), r(bout_d))
            wout_sb = [p2c.tile([128, H], f32, name=f"wo{hp}") for hp in range(4)]
            for hp in range(4):
                nc.sync.dma_start(r(wout_sb[hp]), r(wout_d[hp * 128:(hp + 1) * 128, :]))

            ppool = p2.enter_context(tc.tile_pool(name="pchunks", bufs=12))
            osc_pool = p2.enter_context(tc.tile_pool(name="osc", bufs=2))
            oden_pool = p2.enter_context(tc.tile_pool(name="oden", bufs=8))
            dpool = p2.enter_context(tc.tile_pool(name="dtiles", bufs=2))
            ypool = p2.enter_context(tc.tile_pool(name="ysb", bufs=2))
            ps_s = p2.enter_context(tc.tile_pool(name="ps_s", bufs=2, space="PSUM"))
            ps_o = p2.enter_context(tc.tile_pool(name="ps_o", bufs=2, space="PSUM"))
            ps_y = p2.enter_context(tc.tile_pool(name="ps_y", bufs=2, space="PSUM"))

            def attn_tail(qt, h, opsum, o_dense):
                """softmax denom -> recip -> broadcast -> scale -> dense repack"""
                stage = dpool.tile([65, 512], f32, tag="dstage", name=f"st{qt}_{h}")
                nc.vector.tensor_copy(stage[64:65, :], opsum[64:65, :])
                dp0 = dpool.tile([1, 512], f32, tag="dp0", name=f"dp0_{qt}_{h}")
                nc.sync.dma_start(dp0, stage[64:65, :])
                rp0 = dpool.tile([1, 512], f32, tag="rp0", name=f"rp0_{qt}_{h}")
                nc.vector.reciprocal_approx_fast(rp0, dp0)
                rrep = dpool.tile([64, 512], f32, tag="rrep", name=f"rr{qt}_{h}")
                nc.gpsimd.partition_broadcast(rrep, rp0)
                o_sc = osc_pool.tile([64, 512], f32, tag="osc", name=f"osc{qt}_{h}")
                nc.vector.tensor_mul(o_sc, rrep, opsum[0:64, :])
                p0 = (h % 2) * 64
                nc.sync.dma_start(r(o_dense[p0:p0 + 64, :]), r(o_sc))

            def emit_y_tile(qt, j, ts, oden):
                """one output-projection tile for q-tile qt (b_out via preload)"""
                q0 = qt * 512
                ypsum = ps_y.tile([128, 512], f32, tag="y", name=f"y{qt}_{j}_{ts}")
                nc.tensor.matmul(
                    ypsum, r(ones_sb[0:1, 0:128]),
                    r(bout_sb[0:1, j * 512:(j + 1) * 512]),
                    start=True, stop=False)
                for hp in range(4):
                    nc.tensor.matmul(
                        ypsum,
                        r(oden[hp][:, ts * 128:(ts + 1) * 128]),
                        r(wout_sb[hp][:, j * 512:(j + 1) * 512]),
                        start=False, stop=(hp == 3))
                ysb = ypool.tile([128, 512], f32, tag="ysb", name=f"ys{qt}_{j}_{ts}")
                nc.vector.tensor_copy(ysb, ypsum)
                nc.sync.dma_start(
                    y_d[q0 + ts * 128:q0 + (ts + 1) * 128, j * 512:(j + 1) * 512],
                    ysb)

            pending_y = []   # deferred output-projection tiles of the prev q-tile

            for qt in reversed(range(QT_TILES)):
                q0 = qt * 512
                nk = 4 * (qt + 1)
                oden = []
                for h in range(HPC):
                    if h % 2 == 0:
                        o_dense = oden_pool.tile([128, 512], f32, tag="od",
                                                 name=f"od{qt}_{h // 2}")
                        oden.append(o_dense)
                    hq = qk_sb[h // 2][(h % 2) * 64:(h % 2) * 64 + 64, q0:q0 + 512]
                    # S^T in two-chunk psum tiles, whole-head S stream first
                    # (exp trails on ScalarE with small frequent PE waits that
                    # don't trip the HAM throttle), then the dense PV stream.
                    pts = []
                    for cc in range(nk // 2):
                        spsum = ps_s.tile([128, 1024], f32, tag="s",
                                          name=f"s{qt}_{h}_{cc}")
                        for ci in range(2):
                            c = 2 * cc + ci
                            diag = c >= 4 * qt
                            dd = c - 4 * qt
                            out = spsum[:, ci * 512:(ci + 1) * 512]
                            if diag:
                                nc.tensor.matmul(
                                    out, ident_sb,
                                    cmask_sb[:, dd * 512:(dd + 1) * 512],
                                    start=True, stop=False)
                            hk = qk_sb[4 + h // 2][(h % 2) * 64:(h % 2) * 64 + 64,
                                                   c * 128:(c + 1) * 128]
                            nc.tensor.matmul(out, hk, hq,
                                             start=not diag, stop=True)
                        pt = ppool.tile([128, 1024], bf16, tag="p",
                                        name=f"p{qt}_{h}_{cc}")
                        nc.scalar.activation(pt, spsum, EXP, bias=0.0, scale=1.0)
                        pts.append(pt)
                        if cc == 1 and pending_y:
                            pending_y.pop(0)()
                    opsum = ps_o.tile([65, 512], f32, tag="o", name=f"o{qt}_{h}")
                    for c in range(nk):
                        nc.tensor.matmul(
                            opsum,
                            v_sb[c][:, h * 65:(h + 1) * 65].bitcast(bf16),
                            pts[c // 2][:, (c % 2) * 512:(c % 2) * 512 + 512],
                            start=(c == 0), stop=(c == nk - 1))
                    attn_tail(qt, h, opsum, o_dense)

                for j in range(2):
                    for ts in range(4):
                        pending_y.append(
                            lambda qt=qt, j=j, ts=ts, oden=oden: emit_y_tile(qt, j, ts, oden))

            for fn in pending_y:
                fn()

    nc.compile()
    return nc


_NC_CACHE = None


def _get_nc():
    global _NC_CACHE
    if _NC_CACHE is None:
        _NC_CACHE = _build_nc()
    return _NC_CACHE


def make_core_inputs(input, mask, w_qkv, b_qkv, w_out, b_out, core):
    """Host-side sharding/layout prep for one core."""
    b, g = core // 2, core % 2
    scale = 1.0 / np.sqrt(HD)

    xT = np.ascontiguousarray(input[b].T).astype(np.float32)          # [H, T]

    qcols = slice(g * GD, (g + 1) * GD)
    kcols = slice(H + g * GD, H + (g + 1) * GD)
    vcols = slice(2 * H + g * GD, 2 * H + (g + 1) * GD)
    wq = w_qkv[:, qcols] * scale
    wk = w_qkv[:, kcols]
    wqk = np.ascontiguousarray(np.concatenate([wq, wk], axis=1)).astype(np.float32)
    bqk = np.concatenate([b_qkv[qcols] * scale, b_qkv[kcols]]).astype(np.float32)
    bqkc = np.ascontiguousarray(bqk.reshape(8, 128).T)               # [128, 8]
    wv = np.ascontiguousarray(w_qkv[:, vcols]).astype(np.float32)
    bv = b_qkv[vcols][None, :].astype(np.float32)

    wout = np.ascontiguousarray(w_out[g * GD:(g + 1) * GD, :]).astype(np.float32)
    # b_out on core with g==0 only; zeros on g==1 (partials are summed on host)
    bout = (b_out if g == 0 else np.zeros_like(b_out))[None, :].astype(np.float32)

    padb01 = mask[b].astype(np.float32)                                # [T]
    padb01 = np.ascontiguousarray(padb01.reshape(KC, 128).T)           # [128, KC]

    # 4 causal diagonal mask patterns: delta = 128*dd; valid iff col >= row + delta
    import ml_dtypes
    cm = np.empty((128, 4 * 512), dtype=np.float32)
    rr = np.arange(128)[:, None]
    cc = np.arange(512)[None, :]
    for dd in range(4):
        cm[:, dd * 512:(dd + 1) * 512] = np.where(cc >= rr + 128 * dd, 0.0, NEG)
    cmask = cm.astype(ml_dtypes.bfloat16)
    ident = np.eye(128, dtype=np.float32).astype(ml_dtypes.bfloat16)
    ones = np.ones((1, 512), dtype=np.float32)

    return {
        "xT": xT, "wqk": wqk, "wv": wv, "bqkc": bqkc, "bv": bv,
        "wout": wout, "bout": bout, "padb01": padb01, "cmask": cmask,
        "ident": ident, "ones": ones,
    }


def kernel(input, mask, w_qkv, b_qkv, w_out, b_out):
    from concourse.bass_utils import run_bass_kernel_spmd

    nc = _get_nc()
    in_maps = [
        make_core_inputs(input, mask, w_qkv, b_qkv, w_out, b_out, c)
        for c in range(NCORES)
    ]
    res = run_bass_kernel_spmd(nc, in_maps, list(range(NCORES)))
    parts = [res.results[c]["y"] for c in range(NCORES)]
    out = np.stack([parts[2 * b] + parts[2 * b + 1] for b in range(B)])
    return out.astype(np.float32)


if __name__ == "__main__":
    nc = _build_nc()
    print("build ok")


# revision 21
# speedup vs baseline: 1.3220x; 1.0141x over previous
"""Causal multi-head attention (B=4, T=2048, H=1024, 16 heads) on 8 trn2 cores.

Sharding: batch(4) x head-group(2).  Core c -> batch b=c//2, heads g=c%2
(8 heads each).  Each core computes its QKV projection slice, causal+padding
masked attention for its 8 heads, and a row-parallel slice of the output
projection.  The two partial outputs per batch row are summed on the host
(row-parallel unshard); b_out is folded in via a PSUM preload on one core's
output projection (the other core gets zeros).

Device algorithm (per core, attention kept transposed so softmax reduces
along the PE contraction dim):
  xT [H, T] (host-pretransposed input row)
  QT/KT [512, T] = wqk^T-slices @ xT   (Q pre-scaled by 1/sqrt(hd) on host)
  V    [T, 8x65]  = xT^T @ wv (+bias), bf16, ones column per head; rows with
                    key-padding are zeroed (incl. the ones col) -> padded keys
                    drop out of both the numerator and the softmax denominator.
  per (head pair, q-tile 512, k-chunk pair 2x128):
     S^T[k, q] = KT_h[:, kchunk].T @ QT_h[:, qtile]   (f32r, head pairs packed
                 into PE row groups 0-63 / 64-127 -> concurrent matmuls)
     (+ causal-mask PSUM preload via identity matmul on diagonal chunks)
     P^T = exp(S^T)              (ScalarE, [128,1024] two-chunk ops, bias 0)
     o^T[65, q] += V_aug[kchunk, head].T @ P^T        (row 64 = softmax denom)
  o_scaled = o^T[0:64] * (1/denom)  (DVE approx recip, gpsimd bcast), then
  DMA partition-shift into dense head-pair tiles [128, 512]
  y[t, j] = b_out (preload) + sum_hp o_dense_hp[:, t].T @ wout_hp[:, j]
"""

import os
import sys

import numpy as np

sys.path.insert(0, "/opt/trn_rl_repo")

B, T, H = 4, 2048, 1024
NH, HD = 16, 64
NCORES = 8
HPC = 8          # heads per core
GD = HPC * HD    # head dims per core = 512
KC = T // 128    # 16 k-chunks
QT_TILES = T // 512  # 4 q-tiles
HC = H // 128    # 8 h-chunks (contraction for projections)

NEG = -1.0e9


def _build_nc():
    import concourse.bass as bass
    import concourse.tile as tile
    import concourse.mybir as mybir
    from concourse import bacc
    from contextlib import ExitStack

    f32 = mybir.dt.float32
    f32r = mybir.dt.float32r
    bf16 = mybir.dt.bfloat16
    EXP = mybir.ActivationFunctionType.Exp

    nc = bacc.Bacc("TRN2", target_bir_lowering=False, debug=False)

    xT_d = nc.dram_tensor("xT", [H, T], bf16, kind="ExternalInput").ap()
    wqk_d = nc.dram_tensor("wqk", [H, 2 * GD], bf16, kind="ExternalInput").ap()
    wv_d = nc.dram_tensor("wv", [H, GD], bf16, kind="ExternalInput").ap()
    bqkc_d = nc.dram_tensor("bqkc", [128, 8], f32, kind="ExternalInput").ap()
    bv_d = nc.dram_tensor("bv", [1, GD], f32, kind="ExternalInput").ap()
    wout_d = nc.dram_tensor("wout", [GD, H], f32, kind="ExternalInput").ap()
    bout_d = nc.dram_tensor("bout", [1, H], f32, kind="ExternalInput").ap()
    padb01_d = nc.dram_tensor("padb01", [128, KC], f32, kind="ExternalInput").ap()
    cmask_d = nc.dram_tensor("cmask", [128, 4 * 512], bf16, kind="ExternalInput").ap()
    ident_d = nc.dram_tensor("ident", [128, 128], bf16, kind="ExternalInput").ap()
    ones_d = nc.dram_tensor("ones", [1, 512], f32, kind="ExternalInput").ap()
    y_d = nc.dram_tensor("y", [T, H], f32, kind="ExternalOutput").ap()

    def r(ap):
        return ap.bitcast(f32r)

    with ExitStack() as ctx:
        tc = ctx.enter_context(tile.TileContext(nc))

        const = ctx.enter_context(tc.tile_pool(name="const", bufs=1))
        padb01_sb = const.tile([128, KC], f32, name="padb01_sb")
        nc.sync.dma_start(padb01_sb, padb01_d)
        ones_sb = const.tile([1, 512], f32, name="ones_sb")
        nc.sync.dma_start(r(ones_sb), r(ones_d))

        # Persistent activations
        acts = ctx.enter_context(tc.tile_pool(name="acts", bufs=1))
        qk_sb = [acts.tile([128, T], bf16, name=f"qk{i}") for i in range(8)]
        v_sb = [acts.tile([128, HPC * 65], bf16, name=f"v{c}") for c in range(KC)]

        # ---------------- Phase 1: QKV projections ----------------
        with ExitStack() as p1:
            p1c = p1.enter_context(tc.tile_pool(name="p1c", bufs=1))
            bqkc_sb = p1c.tile([128, 8], f32, name="bqkc_sb")
            nc.sync.dma_start(bqkc_sb, bqkc_d)
            bv_sb = p1c.tile([1, GD], f32, name="bv_sb")
            nc.sync.dma_start(r(bv_sb), r(bv_d))

            xt_pool = p1.enter_context(tc.tile_pool(name="xt", bufs=1))
            xt = [xt_pool.tile([128, T], bf16, name=f"xt{i}") for i in range(HC)]
            for i in range(HC):
                nc.sync.dma_start(xt[i], xT_d[i * 128:(i + 1) * 128, :])

            wqk_pool = p1.enter_context(tc.tile_pool(name="wqkp", bufs=16))
            wv_pool = p1.enter_context(tc.tile_pool(name="wvp", bufs=8))
            ps1 = p1.enter_context(tc.tile_pool(name="ps1", bufs=4, space="PSUM"))

            # Q^T and K^T: out[col, t] tiles
            for ct in range(8):
                wts = []
                for hc in range(HC):
                    wt = wqk_pool.tile([128, 128], bf16, tag="w", name=f"w{ct}_{hc}")
                    nc.sync.dma_start(
                        wt, wqk_d[hc * 128:(hc + 1) * 128, ct * 128:(ct + 1) * 128])
                    wts.append(wt)
                for tt in range(4):
                    ps = ps1.tile([128, 512], f32, tag="ps", name=f"psqk{ct}_{tt}")
                    for hc in range(HC):
                        nc.tensor.matmul(
                            ps, wts[hc], xt[hc][:, tt * 512:(tt + 1) * 512],
                            start=(hc == 0), stop=(hc == HC - 1))
                    nc.vector.tensor_scalar_add(
                        qk_sb[ct][:, tt * 512:(tt + 1) * 512], ps,
                        bqkc_sb[:, ct:ct + 1])

            # V: out[t, col] tiles, bf16, interleaved [8 heads x 65], ones col;
            # all 65 cols multiplied by the key-padding 0/1 mask (per-partition).
            wvts = []
            for hc in range(HC):
                wvt = wv_pool.tile([128, GD], bf16, tag="wv", name=f"wv{hc}")
                nc.sync.dma_start(wvt, wv_d[hc * 128:(hc + 1) * 128, :])
                wvts.append(wvt)
            for ts in range(KC):
                psv = ps1.tile([128, 512], f32, tag="psv", name=f"psv{ts}")
                nc.tensor.matmul(psv, r(ones_sb[0:1, 0:128]), r(bv_sb),
                                 start=True, stop=False)
                for hc in range(HC):
                    nc.tensor.matmul(
                        psv, xt[hc][:, ts * 128:(ts + 1) * 128], wvts[hc],
                        start=False, stop=(hc == HC - 1))
                pad_c = padb01_sb[:, ts:ts + 1]
                dst = v_sb[ts].rearrange("p (h c) -> p h c", h=HPC)[:, :, 0:64]
                src = psv.rearrange("p (h c) -> p h c", h=HPC)
                nc.vector.tensor_scalar_mul(dst, src, pad_c)
                onescols = v_sb[ts].rearrange("p (h c) -> p h c", h=HPC)[:, :, 64:65]
                nc.vector.memset(onescols, 1.0)
                nc.vector.tensor_scalar_mul(onescols, onescols, pad_c)

        # ---------------- Phase 2: attention + output projection ----------------
        with ExitStack() as p2:
            p2c = p2.enter_context(tc.tile_pool(name="p2c", bufs=1))
            ident_sb = p2c.tile([128, 128], bf16, name="ident_sb")
            nc.sync.dma_start(ident_sb, ident_d)
            cmask_sb = p2c.tile([128, 4 * 512], bf16, name="cmask_sb")
            nc.sync.dma_start(cmask_sb, cmask_d)
            bout_sb = p2c.tile([1, H], f32, name="bout_sb")
            nc.sync.dma_start(r(bout_sb), r(bout_d))
            wout_sb = [p2c.tile([128, H], f32, name=f"wo{hp}") for hp in range(4)]
            for hp in range(4):
                nc.sync.dma_start(r(wout_sb[hp]), r(wout_d[hp * 128:(hp + 1) * 128, :]))

            ppool = p2.enter_context(tc.tile_pool(name="pchunks", bufs=12))
            osc_pool = p2.enter_context(tc.tile_pool(name="osc", bufs=2))
            oden_pool = p2.enter_context(tc.tile_pool(name="oden", bufs=8))
            dpool = p2.enter_context(tc.tile_pool(name="dtiles", bufs=2))
            ypool = p2.enter_context(tc.tile_pool(name="ysb", bufs=2))
            ps_s = p2.enter_context(tc.tile_pool(name="ps_s", bufs=2, space="PSUM"))
            ps_o = p2.enter_context(tc.tile_pool(name="ps_o", bufs=2, space="PSUM"))
            ps_y = p2.enter_context(tc.tile_pool(name="ps_y", bufs=2, space="PSUM"))

            def attn_tail(qt, h, opsum, o_dense):
                """softmax denom -> recip -> broadcast -> scale -> dense repack"""
                stage = dpool.tile([65, 512], f32, tag="dstage", name=f"st{qt}_{h}")
                nc.vector.tensor_copy(stage[64:65, :], opsum[64:65, :])
                dp0 = dpool.tile([1, 512], f32, tag="dp0", name=f"dp0_{qt}_{h}")
                nc.sync.dma_start(dp0, stage[64:65, :])
                rp0 = dpool.tile([1, 512], f32, tag="rp0", name=f"rp0_{qt}_{h}")
                nc.vector.reciprocal_approx_fast(rp0, dp0)
                rrep = dpool.tile([64, 512], f32, tag="rrep", name=f"rr{qt}_{h}")
                nc.gpsimd.partition_broadcast(rrep, rp0)
                o_sc = osc_pool.tile([64, 512], f32, tag="osc", name=f"osc{qt}_{h}")
                nc.vector.tensor_mul(o_sc, rrep, opsum[0:64, :])
                p0 = (h % 2) * 64
                nc.sync.dma_start(r(o_dense[p0:p0 + 64, :]), r(o_sc))

            def emit_y_tile(qt, j, ts, oden):
                """one output-projection tile for q-tile qt (b_out via preload)"""
                q0 = qt * 512
                ypsum = ps_y.tile([128, 512], f32, tag="y", name=f"y{qt}_{j}_{ts}")
                nc.tensor.matmul(
                    ypsum, r(ones_sb[0:1, 0:128]),
                    r(bout_sb[0:1, j * 512:(j + 1) * 512]),
                    start=True, stop=False)
                for hp in range(4):
                    nc.tensor.matmul(
                        ypsum,
                        r(oden[hp][:, ts * 128:(ts + 1) * 128]),
                        r(wout_sb[hp][:, j * 512:(j + 1) * 512]),
                        start=False, stop=(hp == 3))
                ysb = ypool.tile([128, 512], f32, tag="ysb", name=f"ys{qt}_{j}_{ts}")
                nc.vector.tensor_copy(ysb, ypsum)
                nc.sync.dma_start(
                    y_d[q0 + ts * 128:q0 + (ts + 1) * 128, j * 512:(j + 1) * 512],
                    ysb)

            pending_y = []   # deferred output-projection tiles of the prev q-tile

            for qt in reversed(range(QT_TILES)):
                q0 = qt * 512
                nk = 4 * (qt + 1)
                oden = []
                for h in range(HPC):
                    if h % 2 == 0:
                        o_dense = oden_pool.tile([128, 512], f32, tag="od",
                                                 name=f"od{qt}_{h // 2}")
                        oden.append(o_dense)
                    hq = qk_sb[h // 2][(h % 2) * 64:(h % 2) * 64 + 64, q0:q0 + 512]
                    # S^T in two-chunk psum tiles, whole-head S stream first
                    # (exp trails on ScalarE with small frequent PE waits that
                    # don't trip the HAM throttle), then the dense PV stream.
                    pts = []
                    for cc in range(nk // 2):
                        spsum = ps_s.tile([128, 1024], f32, tag="s",
                                          name=f"s{qt}_{h}_{cc}")
                        for ci in range(2):
                            c = 2 * cc + ci
                            diag = c >= 4 * qt
                            dd = c - 4 * qt
                            out = spsum[:, ci * 512:(ci + 1) * 512]
                            if diag:
                                nc.tensor.matmul(
                                    out, ident_sb,
                                    cmask_sb[:, dd * 512:(dd + 1) * 512],
                                    start=True, stop=False)
                            hk = qk_sb[4 + h // 2][(h % 2) * 64:(h % 2) * 64 + 64,
                                                   c * 128:(c + 1) * 128]
                            nc.tensor.matmul(out, hk, hq,
                                             start=not diag, stop=True)
                        pt = ppool.tile([128, 1024], bf16, tag="p",
                                        name=f"p{qt}_{h}_{cc}")
                        nc.scalar.activation(pt, spsum, EXP, bias=0.0, scale=1.0)
                        pts.append(pt)
                        if cc == 1 and pending_y:
                            pending_y.pop(0)()
                    opsum = ps_o.tile([65, 512], f32, tag="o", name=f"o{qt}_{h}")
                    for c in range(nk):
                        nc.tensor.matmul(
                            opsum,
                            v_sb[c][:, h * 65:(h + 1) * 65].bitcast(bf16),
                            pts[c // 2][:, (c % 2) * 512:(c % 2) * 512 + 512],
                            start=(c == 0), stop=(c == nk - 1))
                    attn_tail(qt, h, opsum, o_dense)

                for j in range(2):
                    for ts in range(4):
                        pending_y.append(
                            lambda qt=qt, j=j, ts=ts, oden=oden: emit_y_tile(qt, j, ts, oden))

            for fn in pending_y:
                fn()

    nc.compile()
    return nc


_NC_CACHE = None


def _get_nc():
    global _NC_CACHE
    if _NC_CACHE is None:
        _NC_CACHE = _build_nc()
    return _NC_CACHE


def make_core_inputs(input, mask, w_qkv, b_qkv, w_out, b_out, core):
    """Host-side sharding/layout prep for one core."""
    b, g = core // 2, core % 2
    scale = 1.0 / np.sqrt(HD)

    import ml_dtypes
    xT = np.ascontiguousarray(input[b].T).astype(ml_dtypes.bfloat16)  # [H, T]

    qcols = slice(g * GD, (g + 1) * GD)
    kcols = slice(H + g * GD, H + (g + 1) * GD)
    vcols = slice(2 * H + g * GD, 2 * H + (g + 1) * GD)
    wq = w_qkv[:, qcols] * scale
    wk = w_qkv[:, kcols]
    wqk = np.ascontiguousarray(np.concatenate([wq, wk], axis=1)).astype(ml_dtypes.bfloat16)
    bqk = np.concatenate([b_qkv[qcols] * scale, b_qkv[kcols]]).astype(np.float32)
    bqkc = np.ascontiguousarray(bqk.reshape(8, 128).T)               # [128, 8]
    wv = np.ascontiguousarray(w_qkv[:, vcols]).astype(ml_dtypes.bfloat16)
    bv = b_qkv[vcols][None, :].astype(np.float32)

    wout = np.ascontiguousarray(w_out[g * GD:(g + 1) * GD, :]).astype(np.float32)
    # b_out on core with g==0 only; zeros on g==1 (partials are summed on host)
    bout = (b_out if g == 0 else np.zeros_like(b_out))[None, :].astype(np.float32)

    padb01 = mask[b].astype(np.float32)                                # [T]
    padb01 = np.ascontiguousarray(padb01.reshape(KC, 128).T)           # [128, KC]

    # 4 causal diagonal mask patterns: delta = 128*dd; valid iff col >= row + delta
    cm = np.empty((128, 4 * 512), dtype=np.float32)
    rr = np.arange(128)[:, None]
    cc = np.arange(512)[None, :]
    for dd in range(4):
        cm[:, dd * 512:(dd + 1) * 512] = np.where(cc >= rr + 128 * dd, 0.0, NEG)
    cmask = cm.astype(ml_dtypes.bfloat16)
    ident = np.eye(128, dtype=np.float32).astype(ml_dtypes.bfloat16)
    ones = np.ones((1, 512), dtype=np.float32)

    return {
        "xT": xT, "wqk": wqk, "wv": wv, "bqkc": bqkc, "bv": bv,
        "wout": wout, "bout": bout, "padb01": padb01, "cmask": cmask,
        "ident": ident, "ones": ones,
    }


def kernel(input, mask, w_qkv, b_qkv, w_out, b_out):
    from concourse.bass_utils import run_bass_kernel_spmd

    nc = _get_nc()
    in_maps = [
        make_core_inputs(input, mask, w_qkv, b_qkv, w_out, b_out, c)
        for c in range(NCORES)
    ]
    res = run_bass_kernel_spmd(nc, in_maps, list(range(NCORES)))
    parts = [res.results[c]["y"] for c in range(NCORES)]
    out = np.stack([parts[2 * b] + parts[2 * b + 1] for b in range(B)])
    return out.astype(np.float32)


if __name__ == "__main__":
    nc = _build_nc()
    print("build ok")


# revision 22
# speedup vs baseline: 1.3502x; 1.0214x over previous
"""Causal multi-head attention (B=4, T=2048, H=1024, 16 heads) on 8 trn2 cores.

Sharding: batch(4) x head-group(2).  Core c -> batch b=c//2, heads g=c%2
(8 heads each).  Each core computes its QKV projection slice, causal+padding
masked attention for its 8 heads, and a row-parallel slice of the output
projection.  The two partial outputs per batch row are summed on the host
(row-parallel unshard); b_out is folded in via a PSUM preload on one core's
output projection (the other core gets zeros).

Device algorithm (per core, attention kept transposed so softmax reduces
along the PE contraction dim):
  xT [H, T] (host-pretransposed input row)
  QT/KT [512, T] = wqk^T-slices @ xT   (Q pre-scaled by 1/sqrt(hd) on host)
  V    [T, 8x65]  = xT^T @ wv (+bias), bf16, ones column per head; rows with
                    key-padding are zeroed (incl. the ones col) -> padded keys
                    drop out of both the numerator and the softmax denominator.
  per (head pair, q-tile 512, k-chunk pair 2x128):
     S^T[k, q] = KT_h[:, kchunk].T @ QT_h[:, qtile]   (f32r, head pairs packed
                 into PE row groups 0-63 / 64-127 -> concurrent matmuls)
     (+ causal-mask PSUM preload via identity matmul on diagonal chunks)
     P^T = exp(S^T)              (ScalarE, [128,1024] two-chunk ops, bias 0)
     o^T[65, q] += V_aug[kchunk, head].T @ P^T        (row 64 = softmax denom)
  o_scaled = o^T[0:64] * (1/denom)  (DVE approx recip, gpsimd bcast), then
  DMA partition-shift into dense head-pair tiles [128, 512]
  y[t, j] = b_out (preload) + sum_hp o_dense_hp[:, t].T @ wout_hp[:, j]
"""

import os
import sys

import numpy as np

sys.path.insert(0, "/opt/trn_rl_repo")

B, T, H = 4, 2048, 1024
NH, HD = 16, 64
NCORES = 8
HPC = 8          # heads per core
GD = HPC * HD    # head dims per core = 512
KC = T // 128    # 16 k-chunks
QT_TILES = T // 512  # 4 q-tiles
HC = H // 128    # 8 h-chunks (contraction for projections)

NEG = -1.0e9


def _build_nc():
    import concourse.bass as bass
    import concourse.tile as tile
    import concourse.mybir as mybir
    from concourse import bacc
    from contextlib import ExitStack

    f32 = mybir.dt.float32
    f32r = mybir.dt.float32r
    bf16 = mybir.dt.bfloat16
    EXP = mybir.ActivationFunctionType.Exp

    nc = bacc.Bacc("TRN2", target_bir_lowering=False, debug=False)

    xT_d = nc.dram_tensor("xT", [H, T], bf16, kind="ExternalInput").ap()
    wqk_d = nc.dram_tensor("wqk", [H, 2 * GD], bf16, kind="ExternalInput").ap()
    wv_d = nc.dram_tensor("wv", [H, GD], bf16, kind="ExternalInput").ap()
    bqkc_d = nc.dram_tensor("bqkc", [128, 8], f32, kind="ExternalInput").ap()
    bv_d = nc.dram_tensor("bv", [1, GD], f32, kind="ExternalInput").ap()
    wout_d = nc.dram_tensor("wout", [GD, H], f32, kind="ExternalInput").ap()
    bout_d = nc.dram_tensor("bout", [1, H], f32, kind="ExternalInput").ap()
    padb01_d = nc.dram_tensor("padb01", [128, KC], f32, kind="ExternalInput").ap()
    cmask_d = nc.dram_tensor("cmask", [128, 4 * 512], bf16, kind="ExternalInput").ap()
    ones_d = nc.dram_tensor("ones", [1, 512], f32, kind="ExternalInput").ap()
    y_d = nc.dram_tensor("y", [T, H], f32, kind="ExternalOutput").ap()

    def r(ap):
        return ap.bitcast(f32r)

    with ExitStack() as ctx:
        tc = ctx.enter_context(tile.TileContext(nc))

        const = ctx.enter_context(tc.tile_pool(name="const", bufs=1))
        padb01_sb = const.tile([128, KC], f32, name="padb01_sb")
        nc.sync.dma_start(padb01_sb, padb01_d)
        ones_sb = const.tile([1, 512], f32, name="ones_sb")
        nc.sync.dma_start(r(ones_sb), r(ones_d))

        # Persistent activations
        acts = ctx.enter_context(tc.tile_pool(name="acts", bufs=1))
        qk_sb = [acts.tile([128, T], bf16, name=f"qk{i}") for i in range(8)]
        v_sb = [acts.tile([128, HPC * 65], bf16, name=f"v{c}") for c in range(KC)]

        # ---------------- Phase 1: QKV projections ----------------
        with ExitStack() as p1:
            p1c = p1.enter_context(tc.tile_pool(name="p1c", bufs=1))
            bqkc_sb = p1c.tile([128, 8], f32, name="bqkc_sb")
            nc.sync.dma_start(bqkc_sb, bqkc_d)
            bv_sb = p1c.tile([1, GD], f32, name="bv_sb")
            nc.sync.dma_start(r(bv_sb), r(bv_d))

            xt_pool = p1.enter_context(tc.tile_pool(name="xt", bufs=1))
            xt = [xt_pool.tile([128, T], bf16, name=f"xt{i}") for i in range(HC)]
            for i in range(HC):
                nc.sync.dma_start(xt[i], xT_d[i * 128:(i + 1) * 128, :])

            wqk_pool = p1.enter_context(tc.tile_pool(name="wqkp", bufs=16))
            wv_pool = p1.enter_context(tc.tile_pool(name="wvp", bufs=8))
            ps1 = p1.enter_context(tc.tile_pool(name="ps1", bufs=4, space="PSUM"))

            # Q^T and K^T: out[col, t] tiles
            for ct in range(8):
                wts = []
                for hc in range(HC):
                    wt = wqk_pool.tile([128, 128], bf16, tag="w", name=f"w{ct}_{hc}")
                    nc.sync.dma_start(
                        wt, wqk_d[hc * 128:(hc + 1) * 128, ct * 128:(ct + 1) * 128])
                    wts.append(wt)
                for tt in range(4):
                    ps = ps1.tile([128, 512], f32, tag="ps", name=f"psqk{ct}_{tt}")
                    for hc in range(HC):
                        nc.tensor.matmul(
                            ps, wts[hc], xt[hc][:, tt * 512:(tt + 1) * 512],
                            start=(hc == 0), stop=(hc == HC - 1))
                    nc.vector.tensor_scalar_add(
                        qk_sb[ct][:, tt * 512:(tt + 1) * 512], ps,
                        bqkc_sb[:, ct:ct + 1])

            # V: out[t, col] tiles, bf16, interleaved [8 heads x 65], ones col;
            # all 65 cols multiplied by the key-padding 0/1 mask (per-partition).
            wvts = []
            for hc in range(HC):
                wvt = wv_pool.tile([128, GD], bf16, tag="wv", name=f"wv{hc}")
                nc.sync.dma_start(wvt, wv_d[hc * 128:(hc + 1) * 128, :])
                wvts.append(wvt)
            for ts in range(KC):
                psv = ps1.tile([128, 512], f32, tag="psv", name=f"psv{ts}")
                nc.tensor.matmul(psv, r(ones_sb[0:1, 0:128]), r(bv_sb),
                                 start=True, stop=False)
                for hc in range(HC):
                    nc.tensor.matmul(
                        psv, xt[hc][:, ts * 128:(ts + 1) * 128], wvts[hc],
                        start=False, stop=(hc == HC - 1))
                pad_c = padb01_sb[:, ts:ts + 1]
                dst = v_sb[ts].rearrange("p (h c) -> p h c", h=HPC)[:, :, 0:64]
                src = psv.rearrange("p (h c) -> p h c", h=HPC)
                nc.vector.tensor_scalar_mul(dst, src, pad_c)
                onescols = v_sb[ts].rearrange("p (h c) -> p h c", h=HPC)[:, :, 64:65]
                nc.vector.memset(onescols, 1.0)
                nc.vector.tensor_scalar_mul(onescols, onescols, pad_c)

        # ---------------- Phase 2: attention + output projection ----------------
        with ExitStack() as p2:
            p2c = p2.enter_context(tc.tile_pool(name="p2c", bufs=1))
            cmask_sb = p2c.tile([128, 4 * 512], bf16, name="cmask_sb")
            nc.sync.dma_start(cmask_sb, cmask_d)
            bout_sb = p2c.tile([1, H], f32, name="bout_sb")
            nc.sync.dma_start(r(bout_sb), r(bout_d))
            wout_sb = [p2c.tile([128, H], f32, name=f"wo{hp}") for hp in range(4)]
            for hp in range(4):
                nc.sync.dma_start(r(wout_sb[hp]), r(wout_d[hp * 128:(hp + 1) * 128, :]))

            ppool = p2.enter_context(tc.tile_pool(name="pchunks", bufs=12))
            osc_pool = p2.enter_context(tc.tile_pool(name="osc", bufs=2))
            oden_pool = p2.enter_context(tc.tile_pool(name="oden", bufs=8))
            dpool = p2.enter_context(tc.tile_pool(name="dtiles", bufs=2))
            ypool = p2.enter_context(tc.tile_pool(name="ysb", bufs=2))
            ps_s = p2.enter_context(tc.tile_pool(name="ps_s", bufs=2, space="PSUM"))
            ps_o = p2.enter_context(tc.tile_pool(name="ps_o", bufs=2, space="PSUM"))
            ps_y = p2.enter_context(tc.tile_pool(name="ps_y", bufs=2, space="PSUM"))

            def attn_tail(qt, h, opsum, o_dense):
                """softmax denom -> recip -> broadcast -> scale -> dense repack"""
                stage = dpool.tile([65, 512], f32, tag="dstage", name=f"st{qt}_{h}")
                nc.vector.tensor_copy(stage[64:65, :], opsum[64:65, :])
                dp0 = dpool.tile([1, 512], f32, tag="dp0", name=f"dp0_{qt}_{h}")
                nc.sync.dma_start(dp0, stage[64:65, :])
                rp0 = dpool.tile([1, 512], f32, tag="rp0", name=f"rp0_{qt}_{h}")
                nc.vector.reciprocal_approx_fast(rp0, dp0)
                rrep = dpool.tile([64, 512], f32, tag="rrep", name=f"rr{qt}_{h}")
                nc.gpsimd.partition_broadcast(rrep, rp0)
                o_sc = osc_pool.tile([64, 512], f32, tag="osc", name=f"osc{qt}_{h}")
                nc.vector.tensor_mul(o_sc, rrep, opsum[0:64, :])
                p0 = (h % 2) * 64
                nc.sync.dma_start(r(o_dense[p0:p0 + 64, :]), r(o_sc))

            def emit_y_tile(qt, j, ts, oden):
                """one output-projection tile for q-tile qt (b_out via preload)"""
                q0 = qt * 512
                ypsum = ps_y.tile([128, 512], f32, tag="y", name=f"y{qt}_{j}_{ts}")
                nc.tensor.matmul(
                    ypsum, r(ones_sb[0:1, 0:128]),
                    r(bout_sb[0:1, j * 512:(j + 1) * 512]),
                    start=True, stop=False)
                for hp in range(4):
                    nc.tensor.matmul(
                        ypsum,
                        r(oden[hp][:, ts * 128:(ts + 1) * 128]),
                        r(wout_sb[hp][:, j * 512:(j + 1) * 512]),
                        start=False, stop=(hp == 3))
                ysb = ypool.tile([128, 512], f32, tag="ysb", name=f"ys{qt}_{j}_{ts}")
                nc.vector.tensor_copy(ysb, ypsum)
                nc.sync.dma_start(
                    y_d[q0 + ts * 128:q0 + (ts + 1) * 128, j * 512:(j + 1) * 512],
                    ysb)

            pending_y = []   # deferred output-projection tiles of the prev q-tile

            for qt in reversed(range(QT_TILES)):
                q0 = qt * 512
                nk = 4 * (qt + 1)
                oden = []
                for h in range(HPC):
                    if h % 2 == 0:
                        o_dense = oden_pool.tile([128, 512], f32, tag="od",
                                                 name=f"od{qt}_{h // 2}")
                        oden.append(o_dense)
                    hq = qk_sb[h // 2][(h % 2) * 64:(h % 2) * 64 + 64, q0:q0 + 512]
                    # S^T in two-chunk psum tiles, whole-head S stream first
                    # (exp trails on ScalarE with small frequent PE waits that
                    # don't trip the HAM throttle), then the dense PV stream.
                    pts = []
                    for cc in range(nk // 2):
                        spsum = ps_s.tile([128, 1024], f32, tag="s",
                                          name=f"s{qt}_{h}_{cc}")
                        for ci in range(2):
                            c = 2 * cc + ci
                            out = spsum[:, ci * 512:(ci + 1) * 512]
                            hk = qk_sb[4 + h // 2][(h % 2) * 64:(h % 2) * 64 + 64,
                                                   c * 128:(c + 1) * 128]
                            nc.tensor.matmul(out, hk, hq, start=True, stop=True)
                        pt = ppool.tile([128, 1024], bf16, tag="p",
                                        name=f"p{qt}_{h}_{cc}")
                        nc.scalar.activation(pt, spsum, EXP, bias=0.0, scale=1.0)
                        for ci in range(2):
                            c = 2 * cc + ci
                            if c >= 4 * qt:
                                dd = c - 4 * qt
                                sl = pt[:, ci * 512:(ci + 1) * 512]
                                nc.vector.tensor_mul(
                                    sl, cmask_sb[:, dd * 512:(dd + 1) * 512], sl)
                        pts.append(pt)
                        if cc == 1 and pending_y:
                            pending_y.pop(0)()
                    opsum = ps_o.tile([65, 512], f32, tag="o", name=f"o{qt}_{h}")
                    for c in range(nk):
                        nc.tensor.matmul(
                            opsum,
                            v_sb[c][:, h * 65:(h + 1) * 65].bitcast(bf16),
                            pts[c // 2][:, (c % 2) * 512:(c % 2) * 512 + 512],
                            start=(c == 0), stop=(c == nk - 1))
                    attn_tail(qt, h, opsum, o_dense)

                for j in range(2):
                    for ts in range(4):
                        pending_y.append(
                            lambda qt=qt, j=j, ts=ts, oden=oden: emit_y_tile(qt, j, ts, oden))

            for fn in pending_y:
                fn()

    nc.compile()
    return nc


_NC_CACHE = None


def _get_nc():
    global _NC_CACHE
    if _NC_CACHE is None:
        _NC_CACHE = _build_nc()
    return _NC_CACHE


def make_core_inputs(input, mask, w_qkv, b_qkv, w_out, b_out, core):
    """Host-side sharding/layout prep for one core."""
    b, g = core // 2, core % 2
    scale = 1.0 / np.sqrt(HD)

    import ml_dtypes
    xT = np.ascontiguousarray(input[b].T).astype(ml_dtypes.bfloat16)  # [H, T]

    qcols = slice(g * GD, (g + 1) * GD)
    kcols = slice(H + g * GD, H + (g + 1) * GD)
    vcols = slice(2 * H + g * GD, 2 * H + (g + 1) * GD)
    wq = w_qkv[:, qcols] * scale
    wk = w_qkv[:, kcols]
    wqk = np.ascontiguousarray(np.concatenate([wq, wk], axis=1)).astype(ml_dtypes.bfloat16)
    bqk = np.concatenate([b_qkv[qcols] * scale, b_qkv[kcols]]).astype(np.float32)
    bqkc = np.ascontiguousarray(bqk.reshape(8, 128).T)               # [128, 8]
    wv = np.ascontiguousarray(w_qkv[:, vcols]).astype(ml_dtypes.bfloat16)
    bv = b_qkv[vcols][None, :].astype(np.float32)

    wout = np.ascontiguousarray(w_out[g * GD:(g + 1) * GD, :]).astype(np.float32)
    # b_out on core with g==0 only; zeros on g==1 (partials are summed on host)
    bout = (b_out if g == 0 else np.zeros_like(b_out))[None, :].astype(np.float32)

    padb01 = mask[b].astype(np.float32)                                # [T]
    padb01 = np.ascontiguousarray(padb01.reshape(KC, 128).T)           # [128, KC]

    # 4 causal diagonal mask patterns: delta = 128*dd; valid iff col >= row + delta
    cm = np.empty((128, 4 * 512), dtype=np.float32)
    rr = np.arange(128)[:, None]
    cc = np.arange(512)[None, :]
    for dd in range(4):
        cm[:, dd * 512:(dd + 1) * 512] = np.where(cc >= rr + 128 * dd, 1.0, 0.0)
    cmask = cm.astype(ml_dtypes.bfloat16)
    ones = np.ones((1, 512), dtype=np.float32)

    return {
        "xT": xT, "wqk": wqk, "wv": wv, "bqkc": bqkc, "bv": bv,
        "wout": wout, "bout": bout, "padb01": padb01, "cmask": cmask,
        "ones": ones,
    }


def kernel(input, mask, w_qkv, b_qkv, w_out, b_out):
    from concourse.bass_utils import run_bass_kernel_spmd

    nc = _get_nc()
    in_maps = [
        make_core_inputs(input, mask, w_qkv, b_qkv, w_out, b_out, c)
        for c in range(NCORES)
    ]
    res = run_bass_kernel_spmd(nc, in_maps, list(range(NCORES)))
    parts = [res.results[c]["y"] for c in range(NCORES)]
    out = np.stack([parts[2 * b] + parts[2 * b + 1] for b in range(B)])
    return out.astype(np.float32)


if __name__ == "__main__":
    nc = _build_nc()
    print("build ok")


# revision 30
# speedup vs baseline: 1.4767x; 1.0937x over previous
"""Causal multi-head attention (B=4, T=2048, H=1024, 16 heads) on 8 trn2 cores.

Sharding: batch(4) x head-group(2).  Core c -> batch b=c//2, heads g=c%2
(8 heads each).  Each core computes its QKV projection slice, causal+padding
masked attention for its 8 heads, and a row-parallel slice of the output
projection.  The two partial outputs per batch row are summed on the host
(row-parallel unshard); b_out is folded in via a PSUM preload on one core's
output projection (the other core gets zeros).

Device algorithm (per core, attention kept transposed so softmax reduces
along the PE contraction dim):
  xT [H, T] (host-pretransposed input row)
  QT/KT [512, T] = wqk^T-slices @ xT   (Q pre-scaled by 1/sqrt(hd) on host)
  V    [T, 8x65]  = xT^T @ wv (+bias), bf16, ones column per head; rows with
                    key-padding are zeroed (incl. the ones col) -> padded keys
                    drop out of both the numerator and the softmax denominator.
  per (head pair, q-tile 512, k-chunk pair 2x128):
     S^T[k, q] = KT_h[:, kchunk].T @ QT_h[:, qtile]   (f32r, head pairs packed
                 into PE row groups 0-63 / 64-127 -> concurrent matmuls)
     (+ causal-mask PSUM preload via identity matmul on diagonal chunks)
     P^T = exp(S^T)              (ScalarE, [128,1024] two-chunk ops, bias 0)
     o^T[65, q] += V_aug[kchunk, head].T @ P^T        (row 64 = softmax denom)
  o_scaled = o^T[0:64] * (1/denom)  (DVE approx recip, gpsimd bcast), then
  DMA partition-shift into dense head-pair tiles [128, 512]
  y[t, j] = b_out (preload) + sum_hp o_dense_hp[:, t].T @ wout_hp[:, j]
"""

import os
import sys

import numpy as np

sys.path.insert(0, "/opt/trn_rl_repo")

B, T, H = 4, 2048, 1024
NH, HD = 16, 64
NCORES = 8
HPC = 8          # heads per core
GD = HPC * HD    # head dims per core = 512
KC = T // 128    # 16 k-chunks
QT_TILES = T // 512  # 4 q-tiles
HC = H // 128    # 8 h-chunks (contraction for projections)

NEG = -1.0e9


def _build_nc():
    import concourse.bass as bass
    import concourse.tile as tile
    import concourse.mybir as mybir
    from concourse import bacc
    from contextlib import ExitStack

    f32 = mybir.dt.float32
    f32r = mybir.dt.float32r
    bf16 = mybir.dt.bfloat16
    EXP = mybir.ActivationFunctionType.Exp

    nc = bacc.Bacc("TRN2", target_bir_lowering=False, debug=False)

    xT_d = nc.dram_tensor("xT", [H, T], bf16, kind="ExternalInput").ap()
    wqk_d = nc.dram_tensor("wqk", [H, 2 * GD], bf16, kind="ExternalInput").ap()
    wv_d = nc.dram_tensor("wv", [H, GD], bf16, kind="ExternalInput").ap()
    bqkc_d = nc.dram_tensor("bqkc", [128, 8], f32, kind="ExternalInput").ap()
    bv_d = nc.dram_tensor("bv", [1, GD], f32, kind="ExternalInput").ap()
    wout_d = nc.dram_tensor("wout", [GD, H], f32, kind="ExternalInput").ap()
    bout_d = nc.dram_tensor("bout", [1, H], f32, kind="ExternalInput").ap()
    padb01_d = nc.dram_tensor("padb01", [128, KC], f32, kind="ExternalInput").ap()
    cmask_d = nc.dram_tensor("cmask", [128, 4 * 512], bf16, kind="ExternalInput").ap()
    ones_d = nc.dram_tensor("ones", [1, 512], f32, kind="ExternalInput").ap()
    y_d = nc.dram_tensor("y", [T, H], f32, kind="ExternalOutput").ap()

    def r(ap):
        return ap.bitcast(f32r)

    def emit_v_proj(nc, r, ones_sb, bv_sb, xt, wvts, psv, v_sb, padb01_sb,
                    ts, HC, HPC):
        nc.tensor.matmul(psv, r(ones_sb[0:1, 0:128]), r(bv_sb),
                         start=True, stop=False)
        for hc in range(HC):
            nc.tensor.matmul(
                psv, xt[hc][:, ts * 128:(ts + 1) * 128], wvts[hc],
                start=False, stop=(hc == HC - 1))
        pad_c = padb01_sb[:, ts:ts + 1]
        dst = v_sb[ts].rearrange("p (h c) -> p h c", h=HPC)[:, :, 0:64]
        srcv = psv.rearrange("p (h c) -> p h c", h=HPC)
        nc.vector.tensor_scalar_mul(dst, srcv, pad_c)
        onescols = v_sb[ts].rearrange("p (h c) -> p h c", h=HPC)[:, :, 64:65]
        nc.vector.memset(onescols, 1.0)
        nc.vector.tensor_scalar_mul(onescols, onescols, pad_c)

    with ExitStack() as ctx:
        tc = ctx.enter_context(tile.TileContext(nc))

        const = ctx.enter_context(tc.tile_pool(name="const", bufs=1))
        padb01_sb = const.tile([128, KC], f32, name="padb01_sb")
        nc.sync.dma_start(padb01_sb, padb01_d)
        ones_sb = const.tile([1, 512], f32, name="ones_sb")
        nc.sync.dma_start(r(ones_sb), r(ones_d))

        # Persistent activations
        acts = ctx.enter_context(tc.tile_pool(name="acts", bufs=1))
        qk_sb = [acts.tile([128, T], bf16, name=f"qk{i}") for i in range(8)]
        v_sb = [acts.tile([128, HPC * 65], bf16, name=f"v{c}") for c in range(KC)]

        # xt / wv / small consts stay resident through phase 2 (the V
        # projection of chunks 4..15 is braided into the attention stream).
        p1c = ctx.enter_context(tc.tile_pool(name="p1c", bufs=1))
        bqkc_sb = p1c.tile([128, 8], f32, name="bqkc_sb")
        nc.sync.dma_start(bqkc_sb, bqkc_d)
        bv_sb = p1c.tile([1, GD], f32, name="bv_sb")
        nc.sync.dma_start(r(bv_sb), r(bv_d))

        xt_pool = ctx.enter_context(tc.tile_pool(name="xt", bufs=1))
        xt = [xt_pool.tile([128, T], bf16, name=f"xt{i}") for i in range(HC)]
        for tt in range(4):
            for i in range(HC):
                nc.sync.dma_start(xt[i][:, tt * 512:(tt + 1) * 512],
                                  xT_d[i * 128:(i + 1) * 128, tt * 512:(tt + 1) * 512])
        wv_pool = ctx.enter_context(tc.tile_pool(name="wvp", bufs=8))
        wvts = []
        for hc in range(HC):
            wvt = wv_pool.tile([128, GD], bf16, tag="wv", name=f"wv{hc}")
            nc.sync.dma_start(wvt, wv_d[hc * 128:(hc + 1) * 128, :])
            wvts.append(wvt)

        # ---------------- Phase 1: QK projections + V chunks 0..3 ----------
        with ExitStack() as p1:
            wqk_pool = p1.enter_context(tc.tile_pool(name="wqkp", bufs=16))
            ps1 = p1.enter_context(tc.tile_pool(name="ps1", bufs=4, space="PSUM"))

            # Q^T and K^T: out[col, t] tiles
            for ct in range(8):
                wts = []
                for hc in range(HC):
                    wt = wqk_pool.tile([128, 128], bf16, tag="w", name=f"w{ct}_{hc}")
                    nc.sync.dma_start(
                        wt, wqk_d[hc * 128:(hc + 1) * 128, ct * 128:(ct + 1) * 128])
                    wts.append(wt)
                for tt in range(4):
                    ps = ps1.tile([128, 512], f32, tag="ps", name=f"psqk{ct}_{tt}")
                    for hc in range(HC):
                        nc.tensor.matmul(
                            ps, wts[hc], xt[hc][:, tt * 512:(tt + 1) * 512],
                            start=(hc == 0), stop=(hc == HC - 1))
                    nc.vector.tensor_scalar_add(
                        qk_sb[ct][:, tt * 512:(tt + 1) * 512], ps,
                        bqkc_sb[:, ct:ct + 1])

            # V chunks 0..3 (needed by qt0's PV); the rest are braided into
            # the attention stream as PE filler.
            for ts in range(4):
                psv = ps1.tile([128, 512], f32, tag="psv", name=f"psv{ts}")
                emit_v_proj(nc, r, ones_sb, bv_sb, xt, wvts, psv, v_sb,
                            padb01_sb, ts, HC, HPC)

        # ---------------- Phase 2: attention + output projection ----------------
        with ExitStack() as p2:
            p2c = p2.enter_context(tc.tile_pool(name="p2c", bufs=1))
            cmask_sb = p2c.tile([128, 4 * 512], bf16, name="cmask_sb")
            nc.sync.dma_start(cmask_sb, cmask_d)
            bout_sb = p2c.tile([1, H], f32, name="bout_sb")
            nc.sync.dma_start(r(bout_sb), r(bout_d))
            wout_sb = [p2c.tile([128, H], f32, name=f"wo{hp}") for hp in range(4)]
            for hp in range(4):
                nc.sync.dma_start(r(wout_sb[hp]), r(wout_d[hp * 128:(hp + 1) * 128, :]))

            ppool = p2.enter_context(tc.tile_pool(name="pchunks", bufs=12))
            osc_pool = p2.enter_context(tc.tile_pool(name="osc", bufs=2))
            oden_pool = p2.enter_context(tc.tile_pool(name="oden", bufs=8))
            dpool = p2.enter_context(tc.tile_pool(name="dtiles", bufs=2))
            ypool = p2.enter_context(tc.tile_pool(name="ysb", bufs=2))
            ps_s = p2.enter_context(tc.tile_pool(name="ps_s", bufs=2, space="PSUM"))
            ps_o = p2.enter_context(tc.tile_pool(name="ps_o", bufs=2, space="PSUM"))
            ps_y = p2.enter_context(tc.tile_pool(name="ps_y", bufs=2, space="PSUM"))

            def attn_tail(qt, h, opsum, o_dense):
                """softmax denom (row 64) -> recip -> broadcast -> scale -> repack"""
                stage = dpool.tile([65, 512], f32, tag="dstage", name=f"st{qt}_{h}")
                nc.vector.tensor_copy(stage[64:65, :], opsum[64:65, :])
                dp0 = dpool.tile([1, 512], f32, tag="dp0", name=f"dp0_{qt}_{h}")
                nc.sync.dma_start(dp0, stage[64:65, :])
                rp0 = dpool.tile([1, 512], f32, tag="rp0", name=f"rp0_{qt}_{h}")
                nc.vector.reciprocal_approx_fast(rp0, dp0)
                rrep = dpool.tile([64, 512], f32, tag="rrep", name=f"rr{qt}_{h}")
                nc.gpsimd.partition_broadcast(rrep, rp0)
                o_sc = osc_pool.tile([64, 512], f32, tag="osc", name=f"osc{qt}_{h}")
                nc.vector.tensor_mul(o_sc, rrep, opsum[0:64, :])
                p0 = (h % 2) * 64
                nc.sync.dma_start(r(o_dense[p0:p0 + 64, :]), r(o_sc))

            def emit_y_tile(qt, j, ts, oden):
                """one output-projection tile for q-tile qt (b_out via preload)"""
                q0 = qt * 512
                ypsum = ps_y.tile([128, 512], f32, tag="y", name=f"y{qt}_{j}_{ts}")
                nc.tensor.matmul(
                    ypsum, r(ones_sb[0:1, 0:128]),
                    r(bout_sb[0:1, j * 512:(j + 1) * 512]),
                    start=True, stop=False)
                for hp in range(4):
                    nc.tensor.matmul(
                        ypsum,
                        r(oden[hp][:, ts * 128:(ts + 1) * 128]),
                        r(wout_sb[hp][:, j * 512:(j + 1) * 512]),
                        start=False, stop=(hp == 3))
                ysb = ypool.tile([128, 512], f32, tag="ysb", name=f"ys{qt}_{j}_{ts}")
                nc.vector.tensor_copy(ysb, ypsum)
                nc.sync.dma_start(
                    y_d[q0 + ts * 128:q0 + (ts + 1) * 128, j * 512:(j + 1) * 512],
                    ysb)

            pending_y = []   # deferred output-projection tiles of the prev q-tile
            deferred_v = list(range(4, KC))   # V chunks braided as PE filler

            for qt in range(QT_TILES):
                q0 = qt * 512
                nk = 4 * (qt + 1)
                oden = []
                for h in range(HPC):
                    if h % 2 == 0:
                        o_dense = oden_pool.tile([128, 512], f32, tag="od",
                                                 name=f"od{qt}_{h // 2}")
                        oden.append(o_dense)
                    hq = qk_sb[h // 2][(h % 2) * 64:(h % 2) * 64 + 64, q0:q0 + 512]
                    # S^T in two-chunk psum tiles, whole-head S stream first
                    # (exp trails on ScalarE with small frequent PE waits that
                    # don't trip the HAM throttle), then the dense PV stream.
                    pts = []
                    for cc in range(nk // 2):
                        spsum = ps_s.tile([128, 1024], f32, tag="s",
                                          name=f"s{qt}_{h}_{cc}")
                        for ci in range(2):
                            c = 2 * cc + ci
                            out = spsum[:, ci * 512:(ci + 1) * 512]
                            hk = qk_sb[4 + h // 2][(h % 2) * 64:(h % 2) * 64 + 64,
                                                   c * 128:(c + 1) * 128]
                            nc.tensor.matmul(out, hk, hq, start=True, stop=True)
                        pt = ppool.tile([128, 1024], bf16, tag="p",
                                        name=f"p{qt}_{h}_{cc}")
                        nc.scalar.activation(pt, spsum, EXP, bias=0.0, scale=1.0)
                        for ci in range(2):
                            c = 2 * cc + ci
                            if c >= 4 * qt:
                                dd = c - 4 * qt
                                sl = pt[:, ci * 512:(ci + 1) * 512]
                                nc.vector.tensor_mul(
                                    sl, cmask_sb[:, dd * 512:(dd + 1) * 512], sl)
                        pts.append(pt)
                        if deferred_v:
                            ts_v = deferred_v.pop(0)
                            psv = ps_y.tile([128, 512], f32, tag="y",
                                            name=f"psvd{ts_v}")
                            emit_v_proj(nc, r, ones_sb, bv_sb, xt, wvts, psv,
                                        v_sb, padb01_sb, ts_v, HC, HPC)
                        elif cc == 1 and pending_y:
                            pending_y.pop(0)()
                    opsum = ps_o.tile([65, 512], f32, tag="o", name=f"o{qt}_{h}")
                    for c in range(nk):
                        nc.tensor.matmul(
                            opsum,
                            v_sb[c][:, h * 65:(h + 1) * 65].bitcast(bf16),
                            pts[c // 2][:, (c % 2) * 512:(c % 2) * 512 + 512],
                            start=(c == 0), stop=(c == nk - 1))
                    attn_tail(qt, h, opsum, o_dense)

                for j in range(2):
                    for ts in range(4):
                        pending_y.append(
                            lambda qt=qt, j=j, ts=ts, oden=oden: emit_y_tile(qt, j, ts, oden))

            for fn in pending_y:
                fn()

    nc.compile()
    return nc


_NC_CACHE = None


def _get_nc():
    global _NC_CACHE
    if _NC_CACHE is None:
        _NC_CACHE = _build_nc()
    return _NC_CACHE


def make_core_inputs(input, mask, w_qkv, b_qkv, w_out, b_out, core):
    """Host-side sharding/layout prep for one core."""
    b, g = core // 2, core % 2
    scale = 1.0 / np.sqrt(HD)

    import ml_dtypes
    xT = np.ascontiguousarray(input[b].T).astype(ml_dtypes.bfloat16)  # [H, T]

    qcols = slice(g * GD, (g + 1) * GD)
    kcols = slice(H + g * GD, H + (g + 1) * GD)
    vcols = slice(2 * H + g * GD, 2 * H + (g + 1) * GD)
    wq = w_qkv[:, qcols] * scale
    wk = w_qkv[:, kcols]
    wqk = np.ascontiguousarray(np.concatenate([wq, wk], axis=1)).astype(ml_dtypes.bfloat16)
    bqk = np.concatenate([b_qkv[qcols] * scale, b_qkv[kcols]]).astype(np.float32)
    bqkc = np.ascontiguousarray(bqk.reshape(8, 128).T)               # [128, 8]
    wv = np.ascontiguousarray(w_qkv[:, vcols]).astype(ml_dtypes.bfloat16)
    bv = b_qkv[vcols][None, :].astype(np.float32)

    wout = np.ascontiguousarray(w_out[g * GD:(g + 1) * GD, :]).astype(np.float32)
    # b_out on core with g==0 only; zeros on g==1 (partials are summed on host)
    bout = (b_out if g == 0 else np.zeros_like(b_out))[None, :].astype(np.float32)

    padb01 = mask[b].astype(np.float32)                                # [T]
    padb01 = np.ascontiguousarray(padb01.reshape(KC, 128).T)           # [128, KC]

    # 4 causal diagonal mask patterns: delta = 128*dd; valid iff col >= row + delta
    cm = np.empty((128, 4 * 512), dtype=np.float32)
    rr = np.arange(128)[:, None]
    cc = np.arange(512)[None, :]
    for dd in range(4):
        cm[:, dd * 512:(dd + 1) * 512] = np.where(cc >= rr + 128 * dd, 1.0, 0.0)
    cmask = cm.astype(ml_dtypes.bfloat16)
    ones = np.ones((1, 512), dtype=np.float32)

    return {
        "xT": xT, "wqk": wqk, "wv": wv, "bqkc": bqkc, "bv": bv,
        "wout": wout, "bout": bout, "padb01": padb01, "cmask": cmask,
        "ones": ones,
    }


def kernel(input, mask, w_qkv, b_qkv, w_out, b_out):
    from concourse.bass_utils import run_bass_kernel_spmd

    nc = _get_nc()
    in_maps = [
        make_core_inputs(input, mask, w_qkv, b_qkv, w_out, b_out, c)
        for c in range(NCORES)
    ]
    res = run_bass_kernel_spmd(nc, in_maps, list(range(NCORES)))
    parts = [res.results[c]["y"] for c in range(NCORES)]
    out = np.stack([parts[2 * b] + parts[2 * b + 1] for b in range(B)])
    return out.astype(np.float32)


if __name__ == "__main__":
    nc = _build_nc()
    print("build ok")


# revision 31
# speedup vs baseline: 1.5304x; 1.0363x over previous
"""Causal multi-head attention (B=4, T=2048, H=1024, 16 heads) on 8 trn2 cores.

Sharding: batch(4) x head-group(2).  Core c -> batch b=c//2, heads g=c%2
(8 heads each).  Each core computes its QKV projection slice, causal+padding
masked attention for its 8 heads, and a row-parallel slice of the output
projection.  The two partial outputs per batch row are summed on the host
(row-parallel unshard); b_out is folded in via a PSUM preload on one core's
output projection (the other core gets zeros).

Device algorithm (per core, attention kept transposed so softmax reduces
along the PE contraction dim):
  xT [H, T] (host-pretransposed input row)
  QT/KT [512, T] = wqk^T-slices @ xT   (Q pre-scaled by 1/sqrt(hd) on host)
  V    [T, 8x65]  = xT^T @ wv (+bias), bf16, ones column per head; rows with
                    key-padding are zeroed (incl. the ones col) -> padded keys
                    drop out of both the numerator and the softmax denominator.
  per (head pair, q-tile 512, k-chunk pair 2x128):
     S^T[k, q] = KT_h[:, kchunk].T @ QT_h[:, qtile]   (f32r, head pairs packed
                 into PE row groups 0-63 / 64-127 -> concurrent matmuls)
     (+ causal-mask PSUM preload via identity matmul on diagonal chunks)
     P^T = exp(S^T)              (ScalarE, [128,1024] two-chunk ops, bias 0)
     o^T[65, q] += V_aug[kchunk, head].T @ P^T        (row 64 = softmax denom)
  o_scaled = o^T[0:64] * (1/denom)  (DVE approx recip, gpsimd bcast), then
  DMA partition-shift into dense head-pair tiles [128, 512]
  y[t, j] = b_out (preload) + sum_hp o_dense_hp[:, t].T @ wout_hp[:, j]
"""

import os
import sys

import numpy as np

sys.path.insert(0, "/opt/trn_rl_repo")

B, T, H = 4, 2048, 1024
NH, HD = 16, 64
NCORES = 8
HPC = 8          # heads per core
GD = HPC * HD    # head dims per core = 512
KC = T // 128    # 16 k-chunks
QT_TILES = T // 512  # 4 q-tiles
HC = H // 128    # 8 h-chunks (contraction for projections)

NEG = -1.0e9


def _build_nc():
    import concourse.bass as bass
    import concourse.tile as tile
    import concourse.mybir as mybir
    from concourse import bacc
    from contextlib import ExitStack

    f32 = mybir.dt.float32
    f32r = mybir.dt.float32r
    bf16 = mybir.dt.bfloat16
    EXP = mybir.ActivationFunctionType.Exp

    nc = bacc.Bacc("TRN2", target_bir_lowering=False, debug=False)

    xT_d = nc.dram_tensor("xT", [H, T], bf16, kind="ExternalInput").ap()
    wqk_d = nc.dram_tensor("wqk", [H, 2 * GD], bf16, kind="ExternalInput").ap()
    wv_d = nc.dram_tensor("wv", [H, GD], bf16, kind="ExternalInput").ap()
    bqkc_d = nc.dram_tensor("bqkc", [128, 8], f32, kind="ExternalInput").ap()
    bv_d = nc.dram_tensor("bv", [1, GD], f32, kind="ExternalInput").ap()
    wout_d = nc.dram_tensor("wout", [GD, H], f32, kind="ExternalInput").ap()
    bout_d = nc.dram_tensor("bout", [1, H], f32, kind="ExternalInput").ap()
    padb01_d = nc.dram_tensor("padb01", [128, KC], f32, kind="ExternalInput").ap()
    cmask_d = nc.dram_tensor("cmask", [128, 4 * 512], bf16, kind="ExternalInput").ap()
    ones_d = nc.dram_tensor("ones", [1, 512], f32, kind="ExternalInput").ap()
    y_d = nc.dram_tensor("y", [T, H], f32, kind="ExternalOutput").ap()

    def r(ap):
        return ap.bitcast(f32r)

    def emit_v_proj(nc, r, ones_sb, bv_sb, xt, wvts, psv, v_sb, padb01_sb,
                    ts, HC, HPC):
        nc.tensor.matmul(psv, r(ones_sb[0:1, 0:128]), r(bv_sb),
                         start=True, stop=False)
        for hc in range(HC):
            nc.tensor.matmul(
                psv, xt[hc][:, ts * 128:(ts + 1) * 128], wvts[hc],
                start=False, stop=(hc == HC - 1))
        pad_c = padb01_sb[:, ts:ts + 1]
        dst = v_sb[ts].rearrange("p (h c) -> p h c", h=HPC)[:, :, 0:64]
        srcv = psv.rearrange("p (h c) -> p h c", h=HPC)
        nc.vector.tensor_scalar_mul(dst, srcv, pad_c)
        onescols = v_sb[ts].rearrange("p (h c) -> p h c", h=HPC)[:, :, 64:65]
        nc.vector.memset(onescols, 1.0)
        nc.vector.tensor_scalar_mul(onescols, onescols, pad_c)

    with ExitStack() as ctx:
        tc = ctx.enter_context(tile.TileContext(nc))

        const = ctx.enter_context(tc.tile_pool(name="const", bufs=1))
        padb01_sb = const.tile([128, KC], f32, name="padb01_sb")
        nc.sync.dma_start(padb01_sb, padb01_d)
        ones_sb = const.tile([1, 512], f32, name="ones_sb")
        nc.sync.dma_start(r(ones_sb), r(ones_d))

        # Persistent activations
        acts = ctx.enter_context(tc.tile_pool(name="acts", bufs=1))
        qk_sb = [acts.tile([128, T], bf16, name=f"qk{i}") for i in range(8)]
        v_sb = [acts.tile([128, HPC * 65], bf16, name=f"v{c}") for c in range(KC)]

        # xt / wv / small consts stay resident through phase 2 (the V
        # projection of chunks 4..15 is braided into the attention stream).
        p1c = ctx.enter_context(tc.tile_pool(name="p1c", bufs=1))
        bqkc_sb = p1c.tile([128, 8], f32, name="bqkc_sb")
        nc.sync.dma_start(bqkc_sb, bqkc_d)
        bv_sb = p1c.tile([1, GD], f32, name="bv_sb")
        nc.sync.dma_start(r(bv_sb), r(bv_d))

        xt_pool = ctx.enter_context(tc.tile_pool(name="xt", bufs=1))
        xt = [xt_pool.tile([128, T], bf16, name=f"xt{i}") for i in range(HC)]
        for i in range(HC):
            nc.sync.dma_start(xt[i], xT_d[i * 128:(i + 1) * 128, :])
        wv_pool = ctx.enter_context(tc.tile_pool(name="wvp", bufs=8))
        wvts = []
        for hc in range(HC):
            wvt = wv_pool.tile([128, GD], bf16, tag="wv", name=f"wv{hc}")
            nc.sync.dma_start(wvt, wv_d[hc * 128:(hc + 1) * 128, :])
            wvts.append(wvt)

        # ---------------- Phase 1: QK projections + V chunks 0..3 ----------
        with ExitStack() as p1:
            wqk_pool = p1.enter_context(tc.tile_pool(name="wqkp", bufs=16))
            ps1 = p1.enter_context(tc.tile_pool(name="ps1", bufs=4, space="PSUM"))

            # Q^T and K^T: out[col, t] tiles
            for ct in range(8):
                wts = []
                for hc in range(HC):
                    wt = wqk_pool.tile([128, 128], bf16, tag="w", name=f"w{ct}_{hc}")
                    nc.sync.dma_start(
                        wt, wqk_d[hc * 128:(hc + 1) * 128, ct * 128:(ct + 1) * 128])
                    wts.append(wt)
                for tt in range(4):
                    ps = ps1.tile([128, 512], f32, tag="ps", name=f"psqk{ct}_{tt}")
                    for hc in range(HC):
                        nc.tensor.matmul(
                            ps, wts[hc], xt[hc][:, tt * 512:(tt + 1) * 512],
                            start=(hc == 0), stop=(hc == HC - 1))
                    nc.vector.tensor_scalar_add(
                        qk_sb[ct][:, tt * 512:(tt + 1) * 512], ps,
                        bqkc_sb[:, ct:ct + 1])

            # V chunks 0..3 (needed by qt0's PV); the rest are braided into
            # the attention stream as PE filler.
            for ts in range(4):
                psv = ps1.tile([128, 512], f32, tag="psv", name=f"psv{ts}")
                emit_v_proj(nc, r, ones_sb, bv_sb, xt, wvts, psv, v_sb,
                            padb01_sb, ts, HC, HPC)

        # ---------------- Phase 2: attention + output projection ----------------
        with ExitStack() as p2:
            p2c = p2.enter_context(tc.tile_pool(name="p2c", bufs=1))
            cmask_sb = p2c.tile([128, 4 * 512], bf16, name="cmask_sb")
            nc.sync.dma_start(cmask_sb, cmask_d)
            bout_sb = p2c.tile([1, H], f32, name="bout_sb")
            nc.sync.dma_start(r(bout_sb), r(bout_d))
            wout_sb = [p2c.tile([128, H], f32, name=f"wo{hp}") for hp in range(4)]
            for hp in range(4):
                nc.sync.dma_start(r(wout_sb[hp]), r(wout_d[hp * 128:(hp + 1) * 128, :]))

            ppool = p2.enter_context(tc.tile_pool(name="pchunks", bufs=14))
            osc_pool = p2.enter_context(tc.tile_pool(name="osc", bufs=2))
            oden_pool = p2.enter_context(tc.tile_pool(name="oden", bufs=8))
            dpool = p2.enter_context(tc.tile_pool(name="dtiles", bufs=3))
            ypool = p2.enter_context(tc.tile_pool(name="ysb", bufs=3))
            ps_s = p2.enter_context(tc.tile_pool(name="ps_s", bufs=2, space="PSUM"))
            ps_o = p2.enter_context(tc.tile_pool(name="ps_o", bufs=2, space="PSUM"))
            ps_y = p2.enter_context(tc.tile_pool(name="ps_y", bufs=2, space="PSUM"))

            def attn_tail(qt, h, opsum, o_dense):
                """softmax denom -> recip -> broadcast -> scale -> dense repack"""
                stage = dpool.tile([65, 512], f32, tag="dstage", name=f"st{qt}_{h}")
                nc.vector.tensor_copy(stage[64:65, :], opsum[64:65, :])
                dp0 = dpool.tile([1, 512], f32, tag="dp0", name=f"dp0_{qt}_{h}")
                nc.sync.dma_start(dp0, stage[64:65, :])
                rp0 = dpool.tile([1, 512], f32, tag="rp0", name=f"rp0_{qt}_{h}")
                nc.vector.reciprocal_approx_fast(rp0, dp0)
                rrep = dpool.tile([64, 512], f32, tag="rrep", name=f"rr{qt}_{h}")
                nc.gpsimd.partition_broadcast(rrep, rp0)
                if h % 2 == 0:
                    nc.vector.tensor_mul(r(o_dense[0:64, :]), rrep, opsum[0:64, :])
                else:
                    o_sc = osc_pool.tile([64, 512], f32, tag="osc", name=f"osc{qt}_{h}")
                    nc.vector.tensor_mul(o_sc, rrep, opsum[0:64, :])
                    nc.sync.dma_start(r(o_dense[64:128, :]), r(o_sc))

            def emit_y_tile(qt, j, ts, oden):
                """one output-projection tile for q-tile qt (b_out via preload)"""
                q0 = qt * 512
                ypsum = ps_y.tile([128, 512], f32, tag="y", name=f"y{qt}_{j}_{ts}")
                nc.tensor.matmul(
                    ypsum, r(ones_sb[0:1, 0:128]),
                    r(bout_sb[0:1, j * 512:(j + 1) * 512]),
                    start=True, stop=False)
                for hp in range(4):
                    nc.tensor.matmul(
                        ypsum,
                        r(oden[hp][:, ts * 128:(ts + 1) * 128]),
                        r(wout_sb[hp][:, j * 512:(j + 1) * 512]),
                        start=False, stop=(hp == 3))
                ysb = ypool.tile([128, 512], f32, tag="ysb", name=f"ys{qt}_{j}_{ts}")
                nc.vector.tensor_copy(ysb, ypsum)
                nc.sync.dma_start(
                    y_d[q0 + ts * 128:q0 + (ts + 1) * 128, j * 512:(j + 1) * 512],
                    ysb)

            pending_y = []   # deferred output-projection tiles of the prev q-tile
            deferred_v = list(range(4, KC))   # V chunks braided as PE filler

            for qt in range(QT_TILES):
                q0 = qt * 512
                nk = 4 * (qt + 1)
                oden = []
                for h in range(HPC):
                    if h % 2 == 0:
                        o_dense = oden_pool.tile([128, 512], f32, tag="od",
                                                 name=f"od{qt}_{h // 2}")
                        oden.append(o_dense)
                    hq = qk_sb[h // 2][(h % 2) * 64:(h % 2) * 64 + 64, q0:q0 + 512]
                    # S^T in two-chunk psum tiles, whole-head S stream first
                    # (exp trails on ScalarE with small frequent PE waits that
                    # don't trip the HAM throttle), then the dense PV stream.
                    pts = []
                    for cc in range(nk // 2):
                        spsum = ps_s.tile([128, 1024], f32, tag="s",
                                          name=f"s{qt}_{h}_{cc}")
                        for ci in range(2):
                            c = 2 * cc + ci
                            out = spsum[:, ci * 512:(ci + 1) * 512]
                            hk = qk_sb[4 + h // 2][(h % 2) * 64:(h % 2) * 64 + 64,
                                                   c * 128:(c + 1) * 128]
                            nc.tensor.matmul(out, hk, hq, start=True, stop=True)
                        pt = ppool.tile([128, 1024], bf16, tag="p",
                                        name=f"p{qt}_{h}_{cc}")
                        nc.scalar.activation(pt, spsum, EXP, bias=0.0, scale=1.0)
                        for ci in range(2):
                            c = 2 * cc + ci
                            if c >= 4 * qt:
                                dd = c - 4 * qt
                                sl = pt[:, ci * 512:(ci + 1) * 512]
                                nc.vector.tensor_mul(
                                    sl, cmask_sb[:, dd * 512:(dd + 1) * 512], sl)
                        pts.append(pt)
                        if deferred_v:
                            ts_v = deferred_v.pop(0)
                            psv = ps_y.tile([128, 512], f32, tag="y",
                                            name=f"psvd{ts_v}")
                            emit_v_proj(nc, r, ones_sb, bv_sb, xt, wvts, psv,
                                        v_sb, padb01_sb, ts_v, HC, HPC)
                        elif cc == 1 and pending_y:
                            pending_y.pop(0)()
                    opsum = ps_o.tile([65, 512], f32, tag="o", name=f"o{qt}_{h}")
                    for c in range(nk):
                        nc.tensor.matmul(
                            opsum,
                            v_sb[c][:, h * 65:(h + 1) * 65].bitcast(bf16),
                            pts[c // 2][:, (c % 2) * 512:(c % 2) * 512 + 512],
                            start=(c == 0), stop=(c == nk - 1))
                    attn_tail(qt, h, opsum, o_dense)

                for j in range(2):
                    for ts in range(4):
                        pending_y.append(
                            lambda qt=qt, j=j, ts=ts, oden=oden: emit_y_tile(qt, j, ts, oden))

            for fn in pending_y:
                fn()

    nc.compile()
    return nc


_NC_CACHE = None


def _get_nc():
    global _NC_CACHE
    if _NC_CACHE is None:
        _NC_CACHE = _build_nc()
    return _NC_CACHE


def make_core_inputs(input, mask, w_qkv, b_qkv, w_out, b_out, core):
    """Host-side sharding/layout prep for one core."""
    b, g = core // 2, core % 2
    scale = 1.0 / np.sqrt(HD)

    import ml_dtypes
    xT = np.ascontiguousarray(input[b].T).astype(ml_dtypes.bfloat16)  # [H, T]

    qcols = slice(g * GD, (g + 1) * GD)
    kcols = slice(H + g * GD, H + (g + 1) * GD)
    vcols = slice(2 * H + g * GD, 2 * H + (g + 1) * GD)
    wq = w_qkv[:, qcols] * scale
    wk = w_qkv[:, kcols]
    wqk = np.ascontiguousarray(np.concatenate([wq, wk], axis=1)).astype(ml_dtypes.bfloat16)
    bqk = np.concatenate([b_qkv[qcols] * scale, b_qkv[kcols]]).astype(np.float32)
    bqkc = np.ascontiguousarray(bqk.reshape(8, 128).T)               # [128, 8]
    wv = np.ascontiguousarray(w_qkv[:, vcols]).astype(ml_dtypes.bfloat16)
    bv = b_qkv[vcols][None, :].astype(np.float32)

    wout = np.ascontiguousarray(w_out[g * GD:(g + 1) * GD, :]).astype(np.float32)
    # b_out on core with g==0 only; zeros on g==1 (partials are summed on host)
    bout = (b_out if g == 0 else np.zeros_like(b_out))[None, :].astype(np.float32)

    padb01 = mask[b].astype(np.float32)                                # [T]
    padb01 = np.ascontiguousarray(padb01.reshape(KC, 128).T)           # [128, KC]

    # 4 causal diagonal mask patterns: delta = 128*dd; valid iff col >= row + delta
    cm = np.empty((128, 4 * 512), dtype=np.float32)
    rr = np.arange(128)[:, None]
    cc = np.arange(512)[None, :]
    for dd in range(4):
        cm[:, dd * 512:(dd + 1) * 512] = np.where(cc >= rr + 128 * dd, 1.0, 0.0)
    cmask = cm.astype(ml_dtypes.bfloat16)
    ones = np.ones((1, 512), dtype=np.float32)

    return {
        "xT": xT, "wqk": wqk, "wv": wv, "bqkc": bqkc, "bv": bv,
        "wout": wout, "bout": bout, "padb01": padb01, "cmask": cmask,
        "ones": ones,
    }


def kernel(input, mask, w_qkv, b_qkv, w_out, b_out):
    from concourse.bass_utils import run_bass_kernel_spmd

    nc = _get_nc()
    in_maps = [
        make_core_inputs(input, mask, w_qkv, b_qkv, w_out, b_out, c)
        for c in range(NCORES)
    ]
    res = run_bass_kernel_spmd(nc, in_maps, list(range(NCORES)))
    parts = [res.results[c]["y"] for c in range(NCORES)]
    out = np.stack([parts[2 * b] + parts[2 * b + 1] for b in range(B)])
    return out.astype(np.float32)


if __name__ == "__main__":
    nc = _build_nc()
    print("build ok")
